# revision 1
# baseline (speedup 1.0000x reference)
"""Trainium2 Bass kernel for nn_MASNET2 (structure-attention warped resampling).

Pipeline per batch:
  1. axis-max marginals of structure_att  -> x/y profiles
  2. normalize, linear-downsample 448->224, reflect-pad to 670
  3. 447-tap conv (plain + coordinate-weighted) -> smoothed sampling grid
  4. separable bilinear grid-sample of data via two tent-weight matmuls

Sharding: pure data-parallel, batch 64 -> 8 cores x 8.

Implementation notes:
  - grid-sample interpolation matrices are built on-device as tent functions
    relu(1-|y-yc|) = min(max((base+1)-yc,0), max(yc-(base-1),0)) and fed to
    the PE as float32r (fp22) at full rate (N=256 padded moving dim).
  - the 447-tap conv runs as true-fp32 matmuls against a Toeplitz layout of
    filter_w (host-side pure indexing transform).
  - continuous coords are staged through DRAM to broadcast across partitions;
    pad lanes carry -1000 so tent weights vanish there (no memset needed).
"""
import os
import sys

sys.path.insert(0, "/opt/trn_rl_repo")

import numpy as np
from contextlib import ExitStack

import concourse.bass as bass
import concourse.bacc as bacc
import concourse.tile as tile
from concourse import mybir, masks
from concourse.bass_utils import run_bass_kernel_spmd

F32 = mybir.dt.float32
F32R = mybir.dt.float32r
ALU = mybir.AluOpType
ACTF = mybir.ActivationFunctionType

SAM = 224
IN = 448
PAD = 223
GLOB = 670
KSIZE = 447
NCORES = 8
BSH = 8  # batch shard per core

_CACHE = {}

# expose the last run's results for test.py profiling
last_results = None


def _build_program():
    nc = bacc.Bacc("TRN2", num_devices=NCORES)

    data_in = nc.dram_tensor("data", (BSH, 3, IN, IN), F32R, kind="ExternalInput")
    att_in = nc.dram_tensor("att", (BSH, IN, IN), F32, kind="ExternalInput")
    wmat_in = nc.dram_tensor("wmat", (672, SAM), F32, kind="ExternalInput")
    prow_in = nc.dram_tensor("prow", (672,), F32, kind="ExternalInput")
    wrow_in = nc.dram_tensor("wrow", (SAM,), F32, kind="ExternalInput")
    nbp1_in = nc.dram_tensor("nbp1", (112, 4), F32, kind="ExternalInput")
    bm1_in = nc.dram_tensor("bm1", (112, 4), F32, kind="ExternalInput")
    padneg_in = nc.dram_tensor("padneg", (16, 32), F32, kind="ExternalInput")

    out_dram = nc.dram_tensor("out", (BSH, 3, SAM, SAM), F32, kind="ExternalOutput")
    ycst = nc.dram_tensor("ycst", (16, 256), F32, kind="Internal")

    with tile.TileContext(nc) as tc, ExitStack() as ctx:
        consts = ctx.enter_context(tc.tile_pool(name="consts", bufs=1))
        p1pool = ctx.enter_context(tc.tile_pool(name="p1pool", bufs=4))
        sigpool = ctx.enter_context(tc.tile_pool(name="sigpool", bufs=1))
        wpool = ctx.enter_context(tc.tile_pool(name="wpool", bufs=3))
        apool = ctx.enter_context(tc.tile_pool(name="apool", bufs=6))
        epool = ctx.enter_context(tc.tile_pool(name="epool", bufs=3))
        opool = ctx.enter_context(tc.tile_pool(name="opool", bufs=3))
        dpool = ctx.enter_context(tc.tile_pool(name="dpool", bufs=3))
        ps1 = ctx.enter_context(tc.tile_pool(name="ps1", bufs=2, space="PSUM"))
        psA = ctx.enter_context(tc.tile_pool(name="psA", bufs=2, space="PSUM"))
        psB = ctx.enter_context(tc.tile_pool(name="psB", bufs=2, space="PSUM"))

        ident = consts.tile([128, 128], F32)
        masks.make_identity(nc, ident[:])

        nbp1 = consts.tile([112, 4], F32)
        nc.gpsimd.dma_start(out=nbp1, in_=nbp1_in[:, :])
        bm1 = consts.tile([112, 4], F32)
        nc.gpsimd.dma_start(out=bm1, in_=bm1_in[:, :])
        bp1 = consts.tile([112, 4], F32)
        nc.vector.tensor_scalar(out=bp1, in0=nbp1, scalar1=-1.0, scalar2=None,
                                op0=ALU.mult)
        nbm1 = consts.tile([112, 4], F32)
        nc.vector.tensor_scalar(out=nbm1, in0=bm1, scalar1=-1.0, scalar2=None,
                                op0=ALU.mult)
        wrow = consts.tile([16, SAM], F32)
        nc.gpsimd.dma_start(out=wrow, in_=bass.AP(wrow_in, 0, [[0, 16], [1, SAM]]))
        prow = consts.tile([16, 672], F32)
        nc.gpsimd.dma_start(out=prow, in_=bass.AP(prow_in, 0, [[0, 16], [1, 672]]))
        wc = consts.tile([112, 6, SAM], F32)
        nc.gpsimd.dma_start(out=wc, in_=wmat_in.rearrange("(gc p) o -> p gc o", p=112))
        # stage the -1000 pad lanes of ycst once
        pneg = consts.tile([16, 32], F32)
        nc.gpsimd.dma_start(out=pneg, in_=padneg_in[:, :])
        nc.gpsimd.dma_start(out=ycst[:, 224:256], in_=pneg)

        # ---------------- phase 1: marginals for all batches ----------------
        # marg64[p, cc, r] = marginal value at coord cc*112+p for row r
        # r = axis*8 + b   (axis 0 = x-profile from max over y,
        #                   axis 1 = y-profile from max over x)
        marg64 = sigpool.tile([112, 4, 16], F32)
        for b in range(BSH):
            att_t = p1pool.tile([112, 4, IN], F32, tag="att_t")
            nc.sync.dma_start(
                out=att_t, in_=att_in[b].rearrange("(cc p) x -> p cc x", p=112))
            # y-profile: max over x (free dim)
            nc.vector.tensor_reduce(
                out=marg64[:, :, 8 + b], in_=att_t, axis=mybir.AxisListType.X,
                op=ALU.max)
            # x-profile: fold cc by max, transpose, reduce
            m1 = dpool.tile([112, IN], F32, tag="m1")
            nc.vector.tensor_tensor(
                out=m1, in0=att_t[:, 0, :], in1=att_t[:, 1, :], op=ALU.max)
            m2 = dpool.tile([112, IN], F32, tag="m2")
            nc.vector.tensor_tensor(out=m2, in0=att_t[:, 2, :], in1=att_t[:, 3, :],
                                    op=ALU.max)
            nc.vector.tensor_tensor(out=m1, in0=m1, in1=m2, op=ALU.max)
            mt_ps = ps1.tile([112, 4, 112], F32, tag="p1ps")
            for xc in range(4):
                nc.tensor.transpose(
                    mt_ps[:, xc, :], m1[:, xc * 112:(xc + 1) * 112],
                    ident[0:112, 0:112])
            nc.vector.tensor_reduce(
                out=marg64[:, :, b], in_=mt_ps, axis=mybir.AxisListType.X,
                op=ALU.max)

        # reshape marginals to rows: marg16[r, x]
        marg_ps = ps1.tile([16, IN], F32, tag="p1ps")
        for cc in range(4):
            nc.tensor.transpose(
                marg_ps[:, cc * 112:(cc + 1) * 112], marg64[:, cc, :],
                ident[0:112, 0:112])
        marg16 = sigpool.tile([16, IN], F32)
        nc.vector.tensor_copy(out=marg16, in_=marg_ps)

        # ---------------- normalize + interp + pad + P-weight ----------------
        ssum = sigpool.tile([16, 1], F32)
        nc.vector.tensor_reduce(
            out=ssum, in_=marg16, axis=mybir.AxisListType.X, op=ALU.add)
        rsum = sigpool.tile([16, 1], F32)
        nc.vector.reciprocal(out=rsum, in_=ssum)

        even = marg16[:, 0:IN:2]
        odd = marg16[:, 1:IN:2]
        diff = sigpool.tile([16, SAM], F32)
        nc.vector.tensor_tensor(out=diff, in0=odd, in1=even, op=ALU.subtract)
        nc.vector.tensor_tensor(out=diff, in0=diff, in1=wrow, op=ALU.mult)
        msn = sigpool.tile([16, SAM], F32)
        nc.vector.tensor_tensor(out=msn, in0=diff, in1=even, op=ALU.add)

        # sig32 rows 0:16 = normalized padded signal, rows 16:32 = P-weighted
        sig32 = sigpool.tile([48, 672], F32)
        nc.vector.memset(sig32[:, 670:672], 0.0)
        nc.vector.memset(sig32[0:32, :], 0.0)
        nc.scalar.activation(
            out=sig32[0:16, 223:447], in_=msn, func=ACTF.Copy, scale=rsum[:, 0:1])
        rev_l = bass.AP(msn.tensor, msn.offset + 223, [list(msn.ap[0]), [-1, 223]])
        nc.scalar.activation(
            out=sig32[0:16, 0:223], in_=rev_l, func=ACTF.Copy, scale=rsum[:, 0:1])
        rev_r = bass.AP(msn.tensor, msn.offset + 222, [list(msn.ap[0]), [-1, 223]])
        nc.scalar.activation(
            out=sig32[0:16, 447:670], in_=rev_r, func=ACTF.Copy, scale=rsum[:, 0:1])
        nc.vector.tensor_tensor(
            out=sig32[32:48, 0:670], in0=sig32[0:16, 0:670], in1=prow[:, 0:670],
            op=ALU.mult)

        # ---------------- conv via fp32 Toeplitz matmuls ----------------
        sigT_ps = ps1.tile([112, 6, 48], F32, tag="p1ps")
        for gc in range(6):
            nc.tensor.transpose(
                sigT_ps[:, gc, :], sig32[:, gc * 112:(gc + 1) * 112],
                ident[0:48, 0:48])
        sigT = sigpool.tile([112, 6, 48], F32)
        nc.scalar.copy(out=sigT, in_=sigT_ps)
        px_ps = ps1.tile([112, 2, 48], F32, tag="p1ps")
        for oh in range(2):
            for gc in range(6):
                nc.tensor.matmul(
                    px_ps[:, oh, :],
                    lhsT=wc[:, gc, oh * 112:(oh + 1) * 112],
                    rhs=sigT[:, gc, :],
                    start=(gc == 0), stop=(gc == 5))
        px = sigpool.tile([112, 2, 48], F32)
        nc.vector.tensor_copy(out=px, in_=px_ps)

        # xf = conv(P*m)/conv(m); pc = clip(447*xf, 0, 447)
        rec = sigpool.tile([112, 2, 16], F32)
        nc.vector.reciprocal(out=rec, in_=px[:, :, 0:16])
        pc = sigpool.tile([112, 2, 32], F32)
        nc.vector.memset(pc[:, :, 16:32], -1000.0)
        nc.vector.tensor_tensor(
            out=pc[:, :, 0:16], in0=px[:, :, 32:48], in1=rec, op=ALU.mult)
        nc.vector.tensor_scalar(
            out=pc[:, :, 0:16], in0=pc[:, :, 0:16], scalar1=447.0, scalar2=0.0,
            op0=ALU.mult, op1=ALU.max)
        nc.vector.tensor_scalar(
            out=pc[:, :, 0:16], in0=pc[:, :, 0:16], scalar1=447.0, scalar2=None,
            op0=ALU.min)

        # transpose to rows and stage to DRAM
        tr_ps = ps1.tile([64, 112], F32, tag="p1ps")
        nc.tensor.transpose(tr_ps, pc, ident[0:112, 0:112])
        ycr = sigpool.tile([16, SAM], F32)
        nc.vector.tensor_copy(out=ycr[:, 0:112], in_=tr_ps[0:16, :])
        nc.scalar.copy(out=ycr[:, 112:224], in_=tr_ps[32:48, :])
        nc.gpsimd.dma_start(out=ycst[:, 0:224], in_=ycr)

        # broadcast coords to all partitions: ycb[p, r, j]
        ycb = consts.tile([112, 16, 256], F32)
        nc.gpsimd.dma_start(
            out=ycb, in_=bass.AP(ycst, 0, [[0, 112], [256, 16], [1, 256]]))

        # ---------------- phase B: grid-sample ----------------
        for b in range(BSH):
            r_x = b        # x-profile row -> column coords (j)
            r_y = 8 + b    # y-profile row -> row coords (i)
            wy = wpool.tile([112, 4, 256], F32R, tag="wy")
            wx = wpool.tile([112, 4, 256], F32R, tag="wx")
            ycnY = dpool.tile([112, 256], F32, tag="ycnY")
            nc.vector.tensor_scalar(
                out=ycnY, in0=ycb[:, r_y, :], scalar1=-1.0, scalar2=None,
                op0=ALU.mult)

            for cc in range(4):
                uy = dpool.tile([112, 256], F32, tag="uy")
                nc.vector.tensor_scalar(
                    out=uy, in0=ycnY, scalar1=nbp1[:, cc:cc + 1], scalar2=0.0,
                    op0=ALU.subtract, op1=ALU.max)
                vy = dpool.tile([112, 256], F32, tag="vy")
                nc.vector.tensor_scalar(
                    out=vy, in0=ycb[:, r_y, :], scalar1=bm1[:, cc:cc + 1],
                    scalar2=0.0, op0=ALU.subtract, op1=ALU.max)
                nc.vector.tensor_tensor(
                    out=wy[:, cc, :], in0=uy, in1=vy, op=ALU.min)
                ux = dpool.tile([112, 256], F32, tag="ux")
                nc.scalar.activation(
                    out=ux, in_=ycb[:, r_x, :], func=ACTF.Relu,
                    bias=bp1[:, cc:cc + 1], scale=-1.0)
                vx = dpool.tile([112, 256], F32, tag="vx")
                nc.scalar.activation(
                    out=vx, in_=ycb[:, r_x, :], func=ACTF.Relu,
                    bias=nbm1[:, cc:cc + 1], scale=1.0)
                nc.vector.tensor_tensor(
                    out=wx[:, cc, :], in0=ux, in1=vx, op=ALU.min)

            for c in range(3):
                at = apool.tile([112, 4, IN], F32R, tag="at")
                nc.sync.dma_start(
                    out=at, in_=data_in[b, c].rearrange("(cc p) x -> p cc x", p=112))

                bt = epool.tile([112, 4, SAM], F32R, tag="bt")
                for xc in range(4):
                    btp = psA.tile([112, 256], F32, tag="btp")
                    for yc_ in range(4):
                        nc.tensor.matmul(
                            btp, lhsT=at[:, yc_, xc * 112:(xc + 1) * 112],
                            rhs=wy[:, yc_, :],
                            start=(yc_ == 0), stop=(yc_ == 3))
                    if xc % 2 == 0:
                        nc.vector.tensor_copy(out=bt[:, xc, :], in_=btp[:, 0:224])
                    else:
                        nc.scalar.copy(out=bt[:, xc, :], in_=btp[:, 0:224])

                osb = opool.tile([112, 2, SAM], F32, tag="osb")
                for ih in range(2):
                    op = psB.tile([112, 256], F32, tag="op")
                    for xc in range(4):
                        nc.tensor.matmul(
                            op, lhsT=bt[:, xc, ih * 112:(ih + 1) * 112],
                            rhs=wx[:, xc, :],
                            start=(xc == 0), stop=(xc == 3))
                    if ih == 0:
                        nc.vector.tensor_copy(out=osb[:, ih, :], in_=op[:, 0:224])
                    else:
                        nc.scalar.copy(out=osb[:, ih, :], in_=op[:, 0:224])

                nc.scalar.dma_start(
                    out=out_dram[b, c].rearrange("(ih p) j -> p ih j", p=112),
                    in_=osb)
    nc.compile()
    return nc


def _static_consts(filter_w: np.ndarray):
    # Toeplitz layout of the (zero-padded) filter: wmat[g, o] = wpad[223+g-o]
    wpad = np.zeros(896, dtype=np.float32)
    wpad[223:223 + KSIZE] = filter_w
    g = np.arange(672)
    o = np.arange(SAM)
    idx = 223 + g[:, None] - o[None, :]
    valid = (idx >= 0) & (idx < 896)
    wmat = np.zeros((672, SAM), dtype=np.float32)
    wmat[valid] = wpad[idx[valid]]

    prow = np.zeros(672, dtype=np.float32)
    prow[0:GLOB] = (np.arange(GLOB, dtype=np.float32) - PAD) / (SAM - 1.0)
    wrow = (np.arange(SAM, dtype=np.float32) / float(PAD)).astype(np.float32)
    base = (np.arange(112, dtype=np.float32)[:, None]
            + 112.0 * np.arange(4, dtype=np.float32)[None, :])
    nbp1 = (-(base + 1.0)).astype(np.float32)
    bm1 = (base - 1.0).astype(np.float32)
    padneg = np.full((16, 32), -1000.0, dtype=np.float32)
    return {
        "wmat": wmat, "prow": prow, "wrow": wrow,
        "nbp1": nbp1, "bm1": bm1, "padneg": padneg,
    }


def kernel(data: np.ndarray, structure_att: np.ndarray,
           filter_w: np.ndarray) -> np.ndarray:
    global last_results
    data = np.ascontiguousarray(data, dtype=np.float32)
    structure_att = np.ascontiguousarray(structure_att, dtype=np.float32)
    filter_w = np.ascontiguousarray(filter_w, dtype=np.float32)

    if "nc" not in _CACHE:
        _CACHE["nc"] = _build_program()
    nc = _CACHE["nc"]

    consts = _static_consts(filter_w)
    in_maps = []
    for core in range(NCORES):
        sl = slice(core * BSH, (core + 1) * BSH)
        in_maps.append({
            "data": data[sl], "att": structure_att[sl], **consts,
        })

    res = run_bass_kernel_spmd(nc, in_maps, core_ids=list(range(NCORES)))
    last_results = res
    out = np.concatenate([res.results[i]["out"] for i in range(NCORES)], axis=0)
    return out



# revision 12
# speedup vs baseline: 1.4321x; 1.4321x over previous
"""Trainium2 Bass kernel for nn_MASNET2 (structure-attention warped resampling).

Pipeline per batch:
  1. axis-max marginals of structure_att  -> x/y profiles            (f32)
  2. normalize, linear-downsample 448->224, reflect-pad to 670       (f32)
  3. 447-tap conv (plain + coordinate-weighted) -> sampling coords   (f32)
  4. separable bilinear grid-sample of data via two tent matmuls     (fp16)

Sharding: pure data-parallel, batch 64 -> 8 cores x 8.

Key implementation choices:
  - data / tent weights / intermediates / output staged in fp16: halves the
    HBM traffic of the dominant streams and runs the PE at 1 cyc/row with
    no moving-dim padding (N=224). Coordinates stay f32 end-to-end.
  - tents are negated: w = min(|coord - base| - 1, 0) = -tent, one Abs
    (Act engine) + one tensor_scalar (DVE/Pool) per arm; the negations
    cancel across the two matmul stages.
  - coordinate broadcast partition->free via a single PE transpose with a
    stride-0 free-dim access pattern (no DRAM round-trip).
  - marginal/conv chain is split into batch groups (1,1,2,4) so the first
    batch's sampling grid is ready ~12us in and the PE pipeline starts
    while the attention/data DMA stream is still running.
  - all input DMAs are enqueued on the sync queue up front (att first),
    deep apool ring keeps HBM saturated; output writes (one per batch) on
    the Act queue.
"""
import os
import sys

sys.path.insert(0, "/opt/trn_rl_repo")

import numpy as np
from contextlib import ExitStack

import concourse.bass as bass
import concourse.bacc as bacc
import concourse.tile as tile
from concourse import mybir, masks
from concourse.bass_utils import run_bass_kernel_spmd

F32 = mybir.dt.float32
F16 = mybir.dt.float16
ALU = mybir.AluOpType
ACTF = mybir.ActivationFunctionType

SAM = 224
IN = 448
PAD = 223
GLOB = 670
KSIZE = 447
NCORES = 8
BSH = 8  # batch shard per core
GROUPS = [[0], [1], [2, 3], [4, 5], [6, 7]]

_CACHE = {}

# expose the last run's results for test.py profiling
last_results = None


def _flat(t, p_cnt, free_cnt, extra_off=0, stride=1):
    """2D view [p_cnt, free_cnt] of a tile's storage (custom free AP)."""
    return bass.AP(t.tensor, t.offset + extra_off,
                   [[t.ap[0][0], p_cnt], [stride, free_cnt]])


def _bcast_lhsT(t, extra_off, n=112):
    """stride-0 free-dim AP: lhsT[k, m] = t[k]@extra_off for all m."""
    return bass.AP(t.tensor, t.offset + extra_off,
                   [[t.ap[0][0], 112], [0, n]])


def _build_program():
    nc = bacc.Bacc("TRN2", num_devices=NCORES)

    data_in = nc.dram_tensor("data", (BSH, 3, IN, IN), F16, kind="ExternalInput")
    att_in = nc.dram_tensor("att", (BSH, IN, IN), F32, kind="ExternalInput")
    wmat_in = nc.dram_tensor("wmat", (672, SAM), F32, kind="ExternalInput")
    prow_in = nc.dram_tensor("prow", (672,), F32, kind="ExternalInput")
    wrow_in = nc.dram_tensor("wrow", (SAM,), F32, kind="ExternalInput")
    nb_in = nc.dram_tensor("nb", (112, 4), F32, kind="ExternalInput")

    # out[b, c, p, ih, j] = result[b, c, ih*112 + p, j]
    out_dram = nc.dram_tensor("out", (BSH, 3, 112, 2, SAM), F16,
                              kind="ExternalOutput")

    with tile.TileContext(nc) as tc, ExitStack() as ctx:
        consts = ctx.enter_context(tc.tile_pool(name="consts", bufs=1))
        p1pool = ctx.enter_context(tc.tile_pool(name="p1pool", bufs=1))
        dpool = ctx.enter_context(tc.tile_pool(name="dpool", bufs=2))
        sigpool = ctx.enter_context(tc.tile_pool(name="sigpool", bufs=2))
        apool = ctx.enter_context(tc.tile_pool(name="apool", bufs=12))
        wpool = ctx.enter_context(tc.tile_pool(name="wpool", bufs=2))
        epool = ctx.enter_context(tc.tile_pool(name="epool", bufs=3))
        opool = ctx.enter_context(tc.tile_pool(name="opool", bufs=2))
        ps1 = ctx.enter_context(tc.tile_pool(name="ps1", bufs=2, space="PSUM"))
        psA = ctx.enter_context(tc.tile_pool(name="psA", bufs=3, space="PSUM"))
        psB = ctx.enter_context(tc.tile_pool(name="psB", bufs=2, space="PSUM"))
        psC = ctx.enter_context(tc.tile_pool(name="psC", bufs=1, space="PSUM"))

        # ---------------- constants ----------------
        ident = consts.tile([128, 128], F32)
        masks.make_identity(nc, ident[:])
        # ---------------- all input DMAs ----------------
        # att[0] first (split in cc chunks so its marginals start early),
        # then the small constants (Act queue), then att[1..7] and data.
        att_t = []
        for b in range(BSH):
            a = p1pool.tile([112, 4, IN], F32, tag=f"att{b}", bufs=1,
                            name=f"att_t{b}")
            att_t.append(a)
        att0_src = att_in[0].rearrange("(cc p) x -> p cc x", p=112)
        for cc in range(4):
            nc.sync.dma_start(out=att_t[0][:, cc, :], in_=att0_src[:, cc, :])

        nb = consts.tile([112, 4], F32)
        nc.sync.dma_start(out=nb, in_=nb_in[:, :])
        wrow = consts.tile([16, SAM], F32)
        nc.sync.dma_start(out=wrow, in_=bass.AP(wrow_in, 0, [[0, 16], [1, SAM]]))
        prow = consts.tile([16, 672], F32)
        nc.sync.dma_start(out=prow, in_=bass.AP(prow_in, 0, [[0, 16], [1, 672]]))
        wc = consts.tile([112, 6, SAM], F32)
        nc.sync.dma_start(out=wc, in_=wmat_in.rearrange("(gc p) o -> p gc o", p=112))

        for b in range(1, BSH):
            nc.sync.dma_start(
                out=att_t[b], in_=att_in[b].rearrange("(cc p) x -> p cc x", p=112))
        at_tiles = {}
        for b in range(BSH):
            for c in range(3):
                at = apool.tile([112, 4, IN], F16, tag="at", name=f"at{b}{c}")
                nc.sync.dma_start(
                    out=at, in_=data_in[b, c].rearrange("(cc p) x -> p cc x", p=112))
                at_tiles[(b, c)] = at

        # PE p-state warm-up: harmless transposes into the psC slot
        warm = psC.tile([112, 2, SAM], F32, tag="bc", name="warm")
        for _ in range(24):
            nc.tensor.transpose(
                warm[:, 0, 0:112], ident[0:112, 0:112], ident[0:112, 0:112])

        # ---------------- per-group marginals + conv -> coords ----------------
        def emit_group(bs, late=False, gate_ms=None):
            G = len(bs)
            tt_mid = nc.vector.tensor_tensor
            marg = sigpool.tile([112, 4, 8], F32, tag="marg", name="marg")
            gate = (tc.tile_wait_until(gate_ms) if gate_ms is not None
                    else None)
            if gate is not None:
                gate.__enter__()
            for lb, b in enumerate(bs):
                a = att_t[b]
                # y-profile: max over x (free dim), split per cc chunk so
                # the scheduler can interleave critical small ops
                for cc4 in range(4):
                    nc.vector.tensor_reduce(
                        out=marg[:, cc4, G + lb:G + lb + 1], in_=a[:, cc4, :],
                        axis=mybir.AxisListType.X, op=ALU.max)
                # x-profile: fold cc by max (DVE; Pool cannot do max),
                # split in halves for finer scheduling granules
                m1 = dpool.tile([112, IN], F32, tag="m1", name="m1")
                m2 = dpool.tile([112, IN], F32, tag="m2", name="m2")
                for h in range(2):
                    sl = slice(h * 224, (h + 1) * 224)
                    nc.vector.tensor_tensor(
                        out=m1[:, sl], in0=a[:, 0, sl], in1=a[:, 1, sl],
                        op=ALU.max)
                    nc.vector.tensor_tensor(
                        out=m2[:, sl], in0=a[:, 2, sl], in1=a[:, 3, sl],
                        op=ALU.max)
                    nc.vector.tensor_tensor(
                        out=m1[:, sl], in0=m1[:, sl], in1=m2[:, sl],
                        op=ALU.max)
                mt = ps1.tile([112, 4, 112], F32, tag="p1", name="mt")
                for xc in range(4):
                    nc.tensor.transpose(
                        mt[:, xc, :], m1[:, xc * 112:(xc + 1) * 112],
                        ident[0:112, 0:112])
                nc.vector.tensor_reduce(
                    out=marg[:, :, lb], in_=mt, axis=mybir.AxisListType.X,
                    op=ALU.max)
            if gate is not None:
                gate.__exit__(None, None, None)

            # rows 0:G = x-profiles, G:2G = y-profiles
            mgps = ps1.tile([8, 4, 112], F32, tag="p1", name="mgps")
            for cc in range(4):
                nc.tensor.transpose(
                    mgps[0:2 * G, cc, :], marg[:, cc, 0:2 * G],
                    ident[0:112, 0:112])
            mg = sigpool.tile([8, 4, 112], F32, tag="mg", name="mg")
            nc.vector.tensor_copy(
                out=_flat(mg, 2 * G, IN), in_=_flat(mgps, 2 * G, IN))

            # No normalization: the profile scale cancels exactly in
            # xf = conv(P*m)/conv(m).
            # linear downsample 448 -> 224 (align_corners), written straight
            # into the signal tile's center: msn = even + (odd - even) * wrow
            even = _flat(mg, 2 * G, SAM, 0, 2)
            odd = _flat(mg, 2 * G, SAM, 1, 2)
            sig = sigpool.tile([8, 672], F32, tag="sig", name="sig")
            sigP = sigpool.tile([8, 672], F32, tag="sigP", name="sigP")
            nc.gpsimd.memset(sig[0:2 * G, 670:672], 0.0)
            nc.gpsimd.memset(sigP[0:2 * G, 670:672], 0.0)
            diff = sigpool.tile([8, SAM], F32, tag="diff", name="diff")
            tt_mid(out=diff[0:2 * G, :], in0=odd, in1=even, op=ALU.subtract)
            tt_mid(out=diff[0:2 * G, :], in0=diff[0:2 * G, :],
                   in1=wrow[0:2 * G, :], op=ALU.mult)
            tt_mid(out=sig[0:2 * G, 223:447], in0=diff[0:2 * G, :], in1=even,
                   op=ALU.add)
            # reflect pads copied from the center (Pool, SBUF->SBUF)
            lpad = bass.AP(sig.tensor, sig.offset + 446,
                           [[sig.ap[0][0], 2 * G], [-1, 223]])
            nc.gpsimd.tensor_copy(out=sig[0:2 * G, 0:223], in_=lpad)
            rpad = bass.AP(sig.tensor, sig.offset + 445,
                           [[sig.ap[0][0], 2 * G], [-1, 223]])
            nc.gpsimd.tensor_copy(out=sig[0:2 * G, 447:670], in_=rpad)
            tt_mid(out=sigP[0:2 * G, 0:670], in0=sig[0:2 * G, 0:670],
                   in1=prow[0:2 * G, 0:670], op=ALU.mult)

            # transpose signals to [g-part, rows]; cols 0:2G sig, 2G:4G sigP
            sigT_ps = ps1.tile([112, 6, 16], F32, tag="p1", name="sigT_ps")
            for gc in range(6):
                nc.tensor.transpose(
                    sigT_ps[:, gc, 0:2 * G],
                    sig[0:2 * G, gc * 112:(gc + 1) * 112], ident[0:2 * G, 0:2 * G])
                nc.tensor.transpose(
                    sigT_ps[:, gc, 2 * G:4 * G],
                    sigP[0:2 * G, gc * 112:(gc + 1) * 112], ident[0:2 * G, 0:2 * G])
            sigT = sigpool.tile([112, 6, 16], F32, tag="sigT", name="sigT")
            nc.vector.tensor_copy(
                out=bass.AP(sigT.tensor, sigT.offset,
                            [[sigT.ap[0][0], 112], [16, 6], [1, 4 * G]]),
                in_=bass.AP(sigT_ps.tensor, sigT_ps.offset,
                            [[sigT_ps.ap[0][0], 112], [16, 6], [1, 4 * G]]))

            # 447-tap conv via Toeplitz matmuls (true fp32)
            px_ps = ps1.tile([112, 2, 16], F32, tag="p1", name="px_ps")
            for oh in range(2):
                for gc in range(6):
                    nc.tensor.matmul(
                        px_ps[:, oh, 0:4 * G],
                        lhsT=wc[:, gc, oh * 112:(oh + 1) * 112],
                        rhs=sigT[:, gc, 0:4 * G],
                        start=(gc == 0), stop=(gc == 5))
            px = sigpool.tile([112, 2, 16], F32, tag="px", name="px")
            nc.vector.tensor_copy(
                out=bass.AP(px.tensor, px.offset,
                            [[px.ap[0][0], 112], [16, 2], [1, 4 * G]]),
                in_=bass.AP(px_ps.tensor, px_ps.offset,
                            [[px_ps.ap[0][0], 112], [16, 2], [1, 4 * G]]))

            # pc = clip(447 * conv(P*m)/conv(m), 0, 447); col r<G: x, r>=G: y
            rec = sigpool.tile([112, 2, 8], F32, tag="rec", name="rec")
            rec_ap = bass.AP(rec.tensor, rec.offset,
                             [[rec.ap[0][0], 112], [8, 2], [1, 2 * G]])
            nc.vector.reciprocal(
                out=rec_ap,
                in_=bass.AP(px.tensor, px.offset,
                            [[px.ap[0][0], 112], [16, 2], [1, 2 * G]]))
            pc = sigpool.tile([112, 2, 8], F32, tag="pc", name="pc")
            pc_ap = bass.AP(pc.tensor, pc.offset,
                            [[pc.ap[0][0], 112], [8, 2], [1, 2 * G]])
            nc.vector.tensor_tensor(
                out=pc_ap,
                in0=bass.AP(px.tensor, px.offset + 2 * G,
                            [[px.ap[0][0], 112], [16, 2], [1, 2 * G]]),
                in1=rec_ap, op=ALU.mult)
            nc.vector.tensor_scalar(
                out=pc_ap, in0=pc_ap, scalar1=447.0, scalar2=0.0,
                op0=ALU.mult, op1=ALU.max)
            nc.vector.tensor_scalar(
                out=pc_ap, in0=pc_ap, scalar1=447.0, scalar2=None,
                op0=ALU.min)
            return pc

        # per-batch tents from pc (lb = index within group). Returns the
        # weight tiles plus a list of closures that emit the actual ops, so
        # the schedule can interleave them into the previous batch's frames
        # (keeps the Act/DVE/Pool queues free of head-of-line bursts).
        def tent_closures(pc, G, lb):
            # w[p, axis, cc, j]: axis 0 = y tents (rhs of stage 1),
            # axis 1 = x tents (rhs of stage 2); both negated (cancels)
            w = wpool.tile([112, 2, 4, SAM], F16, tag="w", name="w")
            state = {}
            ops = []

            def bc_op():
                # bc[:, 0, :] = y-coords broadcast, bc[:, 1, :] = x-coords
                bc = psC.tile([112, 2, SAM], F32, tag="bc", name="bc")
                for ax, r in ((0, G + lb), (1, lb)):
                    for oh in range(2):
                        nc.tensor.transpose(
                            bc[:, ax, oh * 112:(oh + 1) * 112],
                            _bcast_lhsT(pc, oh * 8 + r), ident[0:112, 0:112])
                state["bc"] = bc

            ops.append(bc_op)
            for cc in range(4):
                def pair(cc=cc):
                    bc = state["bc"]
                    a2 = sigpool.tile([112, 2, SAM], F32, tag="arm", bufs=3,
                                      name="a2")
                    nc.scalar.activation(
                        out=a2, in_=bc, func=ACTF.Abs,
                        bias=nb[:, cc:cc + 1], scale=1.0)
                    nc.gpsimd.tensor_scalar(
                        out=w[:, :, cc, :], in0=a2, scalar1=1.0, scalar2=0.0,
                        op0=ALU.subtract, op1=ALU.min)
                ops.append(pair)
            return w, ops

        def emit_frames(b, w, side):
            osb = opool.tile([112, 3, 2, SAM], F16, tag="osb", name="osb")
            for c in range(3):
                at = at_tiles[(b, c)]
                bt = epool.tile([112, 4, SAM], F16, tag="bt", name="bt")
                for q in range(2):
                    psa = psA.tile([112, 2, SAM], F32, tag="psa", name="psa")
                    for k2 in range(2):
                        xc = 2 * q + k2
                        for yc in range(4):
                            nc.tensor.matmul(
                                psa[:, k2, :],
                                lhsT=at[:, yc, xc * 112:(xc + 1) * 112],
                                rhs=w[:, 0, yc, :],
                                start=(yc == 0), stop=(yc == 3))
                    if q == 0:
                        nc.vector.tensor_copy(out=bt[:, 0:2, :], in_=psa)
                    else:
                        nc.scalar.copy(out=bt[:, 2:4, :], in_=psa)
                psb = psB.tile([112, 2, SAM], F32, tag="psb", name="psb")
                for ih in range(2):
                    for xc in range(4):
                        nc.tensor.matmul(
                            psb[:, ih, :],
                            lhsT=bt[:, xc, ih * 112:(ih + 1) * 112],
                            rhs=w[:, 1, xc, :],
                            start=(xc == 0), stop=(xc == 3))
                nc.scalar.copy(out=osb[:, c, :, :], in_=psb)
                # drain next batch's tent ops, spread over this batch's frames
                take = (len(side) + 2 - c) // (3 - c)
                for _ in range(take):
                    side.popleft()()
            nc.scalar.dma_start(
                out=out_dram[b].rearrange("c p ih j -> p c ih j"), in_=osb)

        # ---------------- interleaved schedule ----------------
        from collections import deque

        group_of = {b: gi for gi, g in enumerate(GROUPS) for b in g}
        lb_of = {b: g.index(b) for g in GROUPS for b in g}
        pcs = {}

        GATES = {3: 0.020, 4: 0.032}

        def ensure_group(gi):
            if gi not in pcs:
                pcs[gi] = emit_group(GROUPS[gi], late=(gi >= 2),
                                     gate_ms=GATES.get(gi))

        w = {}
        ensure_group(0)
        w[0], ops0 = tent_closures(pcs[0], 1, 0)
        for op in ops0:
            op()
        for b in range(BSH):
            nxt = b + 1
            side = deque()
            if nxt < BSH:
                gi = group_of[nxt]
                ensure_group(gi)
                w[nxt], opsn = tent_closures(pcs[gi], len(GROUPS[gi]),
                                             lb_of[nxt])
                side = deque(opsn)
            emit_frames(b, w[b], side)

    nc.compile()
    return nc


def _static_consts(filter_w: np.ndarray):
    # Toeplitz layout of the (zero-padded) filter: wmat[g, o] = wpad[223+g-o]
    wpad = np.zeros(896, dtype=np.float32)
    wpad[223:223 + KSIZE] = filter_w
    g = np.arange(672)
    o = np.arange(SAM)
    idx = 223 + g[:, None] - o[None, :]
    valid = (idx >= 0) & (idx < 896)
    wmat = np.zeros((672, SAM), dtype=np.float32)
    wmat[valid] = wpad[idx[valid]]

    prow = np.zeros(672, dtype=np.float32)
    prow[0:GLOB] = (np.arange(GLOB, dtype=np.float32) - PAD) / (SAM - 1.0)
    wrow = (np.arange(SAM, dtype=np.float32) / float(PAD)).astype(np.float32)
    base = (np.arange(112, dtype=np.float32)[:, None]
            + 112.0 * np.arange(4, dtype=np.float32)[None, :])
    nb = (-base).astype(np.float32)
    return {"wmat": wmat, "prow": prow, "wrow": wrow, "nb": nb}


def kernel(data: np.ndarray, structure_att: np.ndarray,
           filter_w: np.ndarray) -> np.ndarray:
    global last_results
    data16 = np.ascontiguousarray(data, dtype=np.float16)
    structure_att = np.ascontiguousarray(structure_att, dtype=np.float32)
    filter_w = np.ascontiguousarray(filter_w, dtype=np.float32)

    if "nc" not in _CACHE:
        _CACHE["nc"] = _build_program()
    nc = _CACHE["nc"]

    consts = _static_consts(filter_w)
    in_maps = []
    for core in range(NCORES):
        sl = slice(core * BSH, (core + 1) * BSH)
        in_maps.append({
            "data": data16[sl], "att": structure_att[sl], **consts,
        })

    res = run_bass_kernel_spmd(nc, in_maps, core_ids=list(range(NCORES)))
    last_results = res
    parts = []
    for i in range(NCORES):
        o = res.results[i]["out"]  # [BSH, 3, 112, 2, 224] fp16
        parts.append(np.transpose(o, (0, 1, 3, 2, 4)).reshape(BSH, 3, SAM, SAM))
    return np.concatenate(parts, axis=0).astype(np.float32)


# revision 30
# speedup vs baseline: 1.5380x; 1.0740x over previous
"""Trainium2 Bass kernel for nn_MASNET2 (structure-attention warped resampling).

Pipeline per batch:
  1. axis-max marginals of structure_att  -> x/y profiles            (f32)
  2. normalize, linear-downsample 448->224, reflect-pad to 670       (f32)
  3. 447-tap conv (plain + coordinate-weighted) -> sampling coords   (f32)
  4. separable bilinear grid-sample of data via two tent matmuls     (fp16)

Sharding: pure data-parallel, batch 64 -> 8 cores x 8.

Key implementation choices:
  - data / tent weights / intermediates / output staged in fp16: halves the
    HBM traffic of the dominant streams and runs the PE at 1 cyc/row with
    no moving-dim padding (N=224). Coordinates stay f32 end-to-end.
  - tents are negated: w = min(|coord - base| - 1, 0) = -tent, one Abs
    (Act engine) + one tensor_scalar (Pool) per arm; the negations cancel
    across the two matmul stages. The profile normalization is dropped
    entirely (it cancels exactly in conv(P*m)/conv(m)).
  - coordinate broadcast partition->free via a single PE transpose with a
    stride-0 free-dim access pattern (no DRAM round-trip).
  - marginal/conv chain is split into batch groups so the first batch's
    sampling grid is ready ~16us in and the PE pipeline starts while the
    attention/data DMA stream is still running; later groups' marginals
    hide under the frame pipeline.
  - input DMAs are ordered so early batches' data arrives right behind
    their attention frames; PSUM->SBUF copies are spread across DVE/Act
    (GPSIMD cannot touch PSUM), tent arms across Act/Pool.
"""
import sys

sys.path.insert(0, "/opt/trn_rl_repo")

import numpy as np
from contextlib import ExitStack

import concourse.bass as bass
import concourse.bacc as bacc
import concourse.tile as tile
from concourse import mybir, masks
from concourse.bass_utils import run_bass_kernel_spmd

F32 = mybir.dt.float32
F16 = mybir.dt.float16
ALU = mybir.AluOpType
ACTF = mybir.ActivationFunctionType

SAM = 224
IN = 448
PAD = 223
GLOB = 670
KSIZE = 447
NCORES = 8
BSH = 8  # batch shard per core
GROUPS = [[0], [1], [2], [3, 4], [5, 6], [7]]

_CACHE = {}

# expose the last run's results for test.py profiling
last_results = None


def _flat(t, p_cnt, free_cnt, extra_off=0, stride=1):
    """2D view [p_cnt, free_cnt] of a tile's storage (custom free AP)."""
    return bass.AP(t.tensor, t.offset + extra_off,
                   [[t.ap[0][0], p_cnt], [stride, free_cnt]])


def _bcast_lhsT(t, extra_off, n=112):
    """stride-0 free-dim AP: lhsT[k, m] = t[k]@extra_off for all m."""
    return bass.AP(t.tensor, t.offset + extra_off,
                   [[t.ap[0][0], 112], [0, n]])


def _build_program():
    nc = bacc.Bacc("TRN2", num_devices=NCORES)

    data_in = nc.dram_tensor("data", (BSH, 3, IN, IN), F16, kind="ExternalInput")
    att_in = nc.dram_tensor("att", (BSH, IN, IN), F32, kind="ExternalInput")
    wmat_in = nc.dram_tensor("wmat", (672, SAM), F32, kind="ExternalInput")
    prow_in = nc.dram_tensor("prow", (672,), F32, kind="ExternalInput")
    wrow_in = nc.dram_tensor("wrow", (SAM,), F32, kind="ExternalInput")
    nb_in = nc.dram_tensor("nb", (112, 4), F32, kind="ExternalInput")

    # out[b, c, p, ih, j] = result[b, c, ih*112 + p, j]
    out_dram = nc.dram_tensor("out", (BSH, 3, 112, 2, SAM), F16,
                              kind="ExternalOutput")

    with tile.TileContext(nc) as tc, ExitStack() as ctx:
        consts = ctx.enter_context(tc.tile_pool(name="consts", bufs=1))
        p1pool = ctx.enter_context(tc.tile_pool(name="p1pool", bufs=1))
        dpool = ctx.enter_context(tc.tile_pool(name="dpool", bufs=2))
        sigpool = ctx.enter_context(tc.tile_pool(name="sigpool", bufs=2))
        apool = ctx.enter_context(tc.tile_pool(name="apool", bufs=12))
        wpool = ctx.enter_context(tc.tile_pool(name="wpool", bufs=2))
        epool = ctx.enter_context(tc.tile_pool(name="epool", bufs=3))
        opool = ctx.enter_context(tc.tile_pool(name="opool", bufs=2))
        ps1 = ctx.enter_context(tc.tile_pool(name="ps1", bufs=2, space="PSUM"))
        psA = ctx.enter_context(tc.tile_pool(name="psA", bufs=3, space="PSUM"))
        psB = ctx.enter_context(tc.tile_pool(name="psB", bufs=2, space="PSUM"))
        psC = ctx.enter_context(tc.tile_pool(name="psC", bufs=1, space="PSUM"))

        # ---------------- constants ----------------
        ident = consts.tile([128, 128], F32)
        masks.make_identity(nc, ident[:])
        # ---------------- all input DMAs ----------------
        # att[0] first (split in cc chunks so its marginals start early),
        # then the small constants (Act queue), then att[1..7] and data.
        att_t = []
        for b in range(BSH):
            a = p1pool.tile([112, 4, IN], F32, tag=f"att{b}", bufs=1,
                            name=f"att_t{b}")
            att_t.append(a)
        att0_src = att_in[0].rearrange("(cc p) x -> p cc x", p=112)
        for cc in range(4):
            nc.sync.dma_start(out=att_t[0][:, cc, :], in_=att0_src[:, cc, :])

        nb = consts.tile([112, 4], F32)
        nc.sync.dma_start(out=nb, in_=nb_in[:, :])
        wrow = consts.tile([16, SAM], F32)
        nc.sync.dma_start(out=wrow, in_=bass.AP(wrow_in, 0, [[0, 16], [1, SAM]]))
        prow = consts.tile([16, 672], F32)
        nc.sync.dma_start(out=prow, in_=bass.AP(prow_in, 0, [[0, 16], [1, 672]]))
        wc = consts.tile([112, 6, SAM], F32)
        nc.sync.dma_start(out=wc, in_=wmat_in.rearrange("(gc p) o -> p gc o", p=112))

        for b in range(1, BSH):
            nc.sync.dma_start(
                out=att_t[b], in_=att_in[b].rearrange("(cc p) x -> p cc x", p=112))
        at_tiles = {}
        for b in range(BSH):
            for c in range(3):
                at = apool.tile([112, 4, IN], F16, tag="at", name=f"at{b}{c}")
                nc.sync.dma_start(
                    out=at, in_=data_in[b, c].rearrange("(cc p) x -> p cc x", p=112))
                at_tiles[(b, c)] = at

        # ---------------- per-group marginals + conv -> coords ----------------
        def emit_group(bs):
            G = len(bs)
            tt_mid = nc.vector.tensor_tensor
            marg = sigpool.tile([112, 4, 8], F32, tag="marg", name="marg")
            for lb, b in enumerate(bs):
                a = att_t[b]
                # y-profile: max over x (free dim), split per cc chunk so
                # the scheduler can interleave critical small ops
                for cc4 in range(4):
                    nc.vector.tensor_reduce(
                        out=marg[:, cc4, G + lb:G + lb + 1], in_=a[:, cc4, :],
                        axis=mybir.AxisListType.X, op=ALU.max)
                # x-profile: fold cc by max (DVE; Pool cannot do max),
                # split in halves for finer scheduling granules
                m1 = dpool.tile([112, IN], F32, tag="m1", name="m1")
                m2 = dpool.tile([112, IN], F32, tag="m2", name="m2")
                for h in range(2):
                    sl = slice(h * 224, (h + 1) * 224)
                    nc.vector.tensor_tensor(
                        out=m1[:, sl], in0=a[:, 0, sl], in1=a[:, 1, sl],
                        op=ALU.max)
                    nc.vector.tensor_tensor(
                        out=m2[:, sl], in0=a[:, 2, sl], in1=a[:, 3, sl],
                        op=ALU.max)
                    nc.vector.tensor_tensor(
                        out=m1[:, sl], in0=m1[:, sl], in1=m2[:, sl],
                        op=ALU.max)
                mt = ps1.tile([112, 4, 112], F32, tag="p1", name="mt")
                for xc in range(4):
                    nc.tensor.transpose(
                        mt[:, xc, :], m1[:, xc * 112:(xc + 1) * 112],
                        ident[0:112, 0:112])
                nc.vector.tensor_reduce(
                    out=marg[:, :, lb], in_=mt, axis=mybir.AxisListType.X,
                    op=ALU.max)

            # rows 0:G = x-profiles, G:2G = y-profiles
            mgps = ps1.tile([8, 4, 112], F32, tag="p1", name="mgps")
            for cc in range(4):
                nc.tensor.transpose(
                    mgps[0:2 * G, cc, :], marg[:, cc, 0:2 * G],
                    ident[0:112, 0:112])
            mg = sigpool.tile([8, 4, 112], F32, tag="mg", name="mg")
            nc.vector.tensor_copy(
                out=_flat(mg, 2 * G, IN), in_=_flat(mgps, 2 * G, IN))

            # No normalization: the profile scale cancels exactly in
            # xf = conv(P*m)/conv(m).
            # linear downsample 448 -> 224 (align_corners), written straight
            # into the signal tile's center: msn = even + (odd - even) * wrow
            even = _flat(mg, 2 * G, SAM, 0, 2)
            odd = _flat(mg, 2 * G, SAM, 1, 2)
            sig = sigpool.tile([8, 672], F32, tag="sig", name="sig")
            sigP = sigpool.tile([8, 672], F32, tag="sigP", name="sigP")
            nc.gpsimd.memset(sig[0:2 * G, 670:672], 0.0)
            nc.gpsimd.memset(sigP[0:2 * G, 670:672], 0.0)
            diff = sigpool.tile([8, SAM], F32, tag="diff", name="diff")
            tt_mid(out=diff[0:2 * G, :], in0=odd, in1=even, op=ALU.subtract)
            tt_mid(out=diff[0:2 * G, :], in0=diff[0:2 * G, :],
                   in1=wrow[0:2 * G, :], op=ALU.mult)
            tt_mid(out=sig[0:2 * G, 223:447], in0=diff[0:2 * G, :], in1=even,
                   op=ALU.add)
            # reflect pads copied from the center (Pool, SBUF->SBUF)
            lpad = bass.AP(sig.tensor, sig.offset + 446,
                           [[sig.ap[0][0], 2 * G], [-1, 223]])
            nc.gpsimd.tensor_copy(out=sig[0:2 * G, 0:223], in_=lpad)
            rpad = bass.AP(sig.tensor, sig.offset + 445,
                           [[sig.ap[0][0], 2 * G], [-1, 223]])
            nc.gpsimd.tensor_copy(out=sig[0:2 * G, 447:670], in_=rpad)
            tt_mid(out=sigP[0:2 * G, 0:670], in0=sig[0:2 * G, 0:670],
                   in1=prow[0:2 * G, 0:670], op=ALU.mult)

            # transpose signals to [g-part, rows]; cols 0:2G sig, 2G:4G sigP
            sigT_ps = ps1.tile([112, 6, 16], F32, tag="p1", name="sigT_ps")
            for gc in range(6):
                nc.tensor.transpose(
                    sigT_ps[:, gc, 0:2 * G],
                    sig[0:2 * G, gc * 112:(gc + 1) * 112], ident[0:2 * G, 0:2 * G])
                nc.tensor.transpose(
                    sigT_ps[:, gc, 2 * G:4 * G],
                    sigP[0:2 * G, gc * 112:(gc + 1) * 112], ident[0:2 * G, 0:2 * G])
            sigT = sigpool.tile([112, 6, 16], F32, tag="sigT", name="sigT")
            nc.vector.tensor_copy(
                out=bass.AP(sigT.tensor, sigT.offset,
                            [[sigT.ap[0][0], 112], [16, 6], [1, 4 * G]]),
                in_=bass.AP(sigT_ps.tensor, sigT_ps.offset,
                            [[sigT_ps.ap[0][0], 112], [16, 6], [1, 4 * G]]))

            # 447-tap conv via Toeplitz matmuls (true fp32)
            px_ps = ps1.tile([112, 2, 16], F32, tag="p1", name="px_ps")
            for oh in range(2):
                for gc in range(6):
                    nc.tensor.matmul(
                        px_ps[:, oh, 0:4 * G],
                        lhsT=wc[:, gc, oh * 112:(oh + 1) * 112],
                        rhs=sigT[:, gc, 0:4 * G],
                        start=(gc == 0), stop=(gc == 5))
            px = sigpool.tile([112, 2, 16], F32, tag="px", name="px")
            nc.vector.tensor_copy(
                out=bass.AP(px.tensor, px.offset,
                            [[px.ap[0][0], 112], [16, 2], [1, 4 * G]]),
                in_=bass.AP(px_ps.tensor, px_ps.offset,
                            [[px_ps.ap[0][0], 112], [16, 2], [1, 4 * G]]))

            # pc = clip(447 * conv(P*m)/conv(m), 0, 447); col r<G: x, r>=G: y
            rec = sigpool.tile([112, 2, 8], F32, tag="rec", name="rec")
            rec_ap = bass.AP(rec.tensor, rec.offset,
                             [[rec.ap[0][0], 112], [8, 2], [1, 2 * G]])
            nc.vector.reciprocal(
                out=rec_ap,
                in_=bass.AP(px.tensor, px.offset,
                            [[px.ap[0][0], 112], [16, 2], [1, 2 * G]]))
            pc = sigpool.tile([112, 2, 8], F32, tag="pc", name="pc")
            pc_ap = bass.AP(pc.tensor, pc.offset,
                            [[pc.ap[0][0], 112], [8, 2], [1, 2 * G]])
            nc.vector.tensor_tensor(
                out=pc_ap,
                in0=bass.AP(px.tensor, px.offset + 2 * G,
                            [[px.ap[0][0], 112], [16, 2], [1, 2 * G]]),
                in1=rec_ap, op=ALU.mult)
            nc.vector.tensor_scalar(
                out=pc_ap, in0=pc_ap, scalar1=447.0, scalar2=0.0,
                op0=ALU.mult, op1=ALU.max)
            nc.vector.tensor_scalar(
                out=pc_ap, in0=pc_ap, scalar1=447.0, scalar2=None,
                op0=ALU.min)
            return pc

        # per-batch tents from pc (lb = index within group). Returns the
        # weight tiles plus a list of closures that emit the actual ops, so
        # the schedule can interleave them into the previous batch's frames
        # (keeps the Act/DVE/Pool queues free of head-of-line bursts).
        def tent_closures(pc, G, lb):
            # w[p, axis, cc, j]: axis 0 = y tents (rhs of stage 1),
            # axis 1 = x tents (rhs of stage 2); both negated (cancels)
            w = wpool.tile([112, 2, 4, SAM], F16, tag="w", name="w")
            state = {}
            ops = []

            def bc_op():
                # bc[:, 0, :] = y-coords broadcast, bc[:, 1, :] = x-coords
                bc = psC.tile([112, 2, SAM], F32, tag="bc", name="bc")
                for ax, r in ((0, G + lb), (1, lb)):
                    for oh in range(2):
                        nc.tensor.transpose(
                            bc[:, ax, oh * 112:(oh + 1) * 112],
                            _bcast_lhsT(pc, oh * 8 + r), ident[0:112, 0:112])
                state["bc"] = bc

            ops.append(bc_op)
            # y tents first: stage-1's accumulation chain consumes them in
            # cc order, x tents only gate stage-2
            for ax in range(2):
                for cc in range(4):
                    def pair(ax=ax, cc=cc):
                        bc = state["bc"]
                        a2 = sigpool.tile([112, SAM], F32, tag="arm", bufs=3,
                                          name="a2")
                        nc.scalar.activation(
                            out=a2, in_=bc[:, ax, :], func=ACTF.Abs,
                            bias=nb[:, cc:cc + 1], scale=1.0)
                        nc.gpsimd.tensor_scalar(
                            out=w[:, ax, cc, :], in0=a2, scalar1=1.0,
                            scalar2=0.0, op0=ALU.subtract, op1=ALU.min)
                    ops.append(pair)
            return w, ops

        def emit_frames(b, w, side):
            late_b = b >= 4
            osb = opool.tile([112, 3, 2, SAM], F16, tag="osb", name="osb")
            for c in range(3):
                at = at_tiles[(b, c)]
                bt = epool.tile([112, 4, SAM], F16, tag="bt", name="bt")
                for q in range(2):
                    psa = psA.tile([112, 2, SAM], F32, tag="psa", name="psa")
                    for k2 in range(2):
                        xc = 2 * q + k2
                        for yc in range(4):
                            nc.tensor.matmul(
                                psa[:, k2, :],
                                lhsT=at[:, yc, xc * 112:(xc + 1) * 112],
                                rhs=w[:, 0, yc, :],
                                start=(yc == 0), stop=(yc == 3))
                    if late_b and q == 0:
                        nc.vector.tensor_copy(out=bt[:, 0:2, :], in_=psa)
                    else:
                        nc.scalar.copy(out=bt[:, 2 * q:2 * q + 2, :], in_=psa)
                if side and c == 1:
                    side.popleft()()
                psb = psB.tile([112, 2, SAM], F32, tag="psb", name="psb")
                for ih in range(2):
                    for xc in range(4):
                        nc.tensor.matmul(
                            psb[:, ih, :],
                            lhsT=bt[:, xc, ih * 112:(ih + 1) * 112],
                            rhs=w[:, 1, xc, :],
                            start=(xc == 0), stop=(xc == 3))
                if late_b:
                    nc.vector.tensor_copy(out=osb[:, c, :, :], in_=psb)
                else:
                    nc.scalar.copy(out=osb[:, c, :, :], in_=psb)
                if b == BSH - 1:
                    # split the final batch's writes per frame: shortens the
                    # end-of-program tail
                    nc.scalar.dma_start(out=out_dram[b, c], in_=osb[:, c, :, :])
                # drain next batch's tent ops, spread over this batch's frames
                take = (len(side) + 2 - c) // (3 - c)
                for _ in range(take):
                    side.popleft()()
            if b < BSH - 1:
                nc.scalar.dma_start(
                    out=out_dram[b].rearrange("c p ih j -> p c ih j"),
                    in_=osb)

        # ---------------- interleaved schedule ----------------
        from collections import deque

        group_of = {b: gi for gi, g in enumerate(GROUPS) for b in g}
        lb_of = {b: g.index(b) for g in GROUPS for b in g}
        pcs = {}

        def ensure_group(gi):
            if gi not in pcs:
                pcs[gi] = emit_group(GROUPS[gi])

        w = {}
        ensure_group(0)
        w[0], ops0 = tent_closures(pcs[0], 1, 0)
        for op in ops0:
            op()
        for b in range(BSH):
            nxt = b + 1
            side = deque()
            if nxt < BSH:
                gi = group_of[nxt]
                ensure_group(gi)
                w[nxt], opsn = tent_closures(pcs[gi], len(GROUPS[gi]),
                                             lb_of[nxt])
                side = deque(opsn)
            emit_frames(b, w[b], side)

    nc.compile()
    return nc


def _static_consts(filter_w: np.ndarray):
    # Toeplitz layout of the (zero-padded) filter: wmat[g, o] = wpad[223+g-o]
    wpad = np.zeros(896, dtype=np.float32)
    wpad[223:223 + KSIZE] = filter_w
    g = np.arange(672)
    o = np.arange(SAM)
    idx = 223 + g[:, None] - o[None, :]
    valid = (idx >= 0) & (idx < 896)
    wmat = np.zeros((672, SAM), dtype=np.float32)
    wmat[valid] = wpad[idx[valid]]

    prow = np.zeros(672, dtype=np.float32)
    prow[0:GLOB] = (np.arange(GLOB, dtype=np.float32) - PAD) / (SAM - 1.0)
    wrow = (np.arange(SAM, dtype=np.float32) / float(PAD)).astype(np.float32)
    base = (np.arange(112, dtype=np.float32)[:, None]
            + 112.0 * np.arange(4, dtype=np.float32)[None, :])
    nb = (-base).astype(np.float32)
    return {"wmat": wmat, "prow": prow, "wrow": wrow, "nb": nb}


def kernel(data: np.ndarray, structure_att: np.ndarray,
           filter_w: np.ndarray) -> np.ndarray:
    global last_results
    data16 = np.ascontiguousarray(data, dtype=np.float16)
    structure_att = np.ascontiguousarray(structure_att, dtype=np.float32)
    filter_w = np.ascontiguousarray(filter_w, dtype=np.float32)

    if "nc" not in _CACHE:
        _CACHE["nc"] = _build_program()
    nc = _CACHE["nc"]

    consts = _static_consts(filter_w)
    in_maps = []
    for core in range(NCORES):
        sl = slice(core * BSH, (core + 1) * BSH)
        in_maps.append({
            "data": data16[sl], "att": structure_att[sl], **consts,
        })

    res = run_bass_kernel_spmd(nc, in_maps, core_ids=list(range(NCORES)))
    last_results = res
    parts = []
    for i in range(NCORES):
        o = res.results[i]["out"]  # [BSH, 3, 112, 2, 224] fp16
        parts.append(np.transpose(o, (0, 1, 3, 2, 4)).reshape(BSH, 3, SAM, SAM))
    return np.concatenate(parts, axis=0).astype(np.float32)


# revision 31
# speedup vs baseline: 1.5403x; 1.0015x over previous
"""Trainium2 Bass kernel for nn_MASNET2 (structure-attention warped resampling).

Pipeline per batch:
  1. axis-max marginals of structure_att  -> x/y profiles            (f32)
  2. normalize, linear-downsample 448->224, reflect-pad to 670       (f32)
  3. 447-tap conv (plain + coordinate-weighted) -> sampling coords   (f32)
  4. separable bilinear grid-sample of data via two tent matmuls     (fp16)

Sharding: pure data-parallel, batch 64 -> 8 cores x 8.

Key implementation choices:
  - data / tent weights / intermediates / output staged in fp16: halves the
    HBM traffic of the dominant streams and runs the PE at 1 cyc/row with
    no moving-dim padding (N=224). Coordinates stay f32 end-to-end.
  - tents are negated: w = min(|coord - base| - 1, 0) = -tent, one Abs
    (Act engine) + one tensor_scalar (Pool) per arm; the negations cancel
    across the two matmul stages. The profile normalization is dropped
    entirely (it cancels exactly in conv(P*m)/conv(m)).
  - coordinate broadcast partition->free via a single PE transpose with a
    stride-0 free-dim access pattern (no DRAM round-trip).
  - marginal/conv chain is split into batch groups so the first batch's
    sampling grid is ready ~16us in and the PE pipeline starts while the
    attention/data DMA stream is still running; later groups' marginals
    hide under the frame pipeline.
  - input DMAs are ordered so early batches' data arrives right behind
    their attention frames; PSUM->SBUF copies are spread across DVE/Act
    (GPSIMD cannot touch PSUM), tent arms across Act/Pool.
"""
import sys

sys.path.insert(0, "/opt/trn_rl_repo")

import numpy as np
from contextlib import ExitStack

import concourse.bass as bass
import concourse.bacc as bacc
import concourse.tile as tile
from concourse import mybir, masks
from concourse.bass_utils import run_bass_kernel_spmd

F32 = mybir.dt.float32
F16 = mybir.dt.float16
ALU = mybir.AluOpType
ACTF = mybir.ActivationFunctionType

SAM = 224
IN = 448
PAD = 223
GLOB = 670
KSIZE = 447
NCORES = 8
BSH = 8  # batch shard per core
GROUPS = [[0], [1], [2], [3, 4], [5, 6], [7]]

_CACHE = {}

# expose the last run's results for test.py profiling
last_results = None


def _flat(t, p_cnt, free_cnt, extra_off=0, stride=1):
    """2D view [p_cnt, free_cnt] of a tile's storage (custom free AP)."""
    return bass.AP(t.tensor, t.offset + extra_off,
                   [[t.ap[0][0], p_cnt], [stride, free_cnt]])


def _bcast_lhsT(t, extra_off, n=112):
    """stride-0 free-dim AP: lhsT[k, m] = t[k]@extra_off for all m."""
    return bass.AP(t.tensor, t.offset + extra_off,
                   [[t.ap[0][0], 112], [0, n]])


def _build_program():
    nc = bacc.Bacc("TRN2", num_devices=NCORES)

    data_in = nc.dram_tensor("data", (BSH, 3, IN, IN), F16, kind="ExternalInput")
    att_in = nc.dram_tensor("att", (BSH, IN, IN), F32, kind="ExternalInput")
    wmat_in = nc.dram_tensor("wmat", (672, SAM), F32, kind="ExternalInput")
    prow_in = nc.dram_tensor("prow", (672,), F32, kind="ExternalInput")
    wrow_in = nc.dram_tensor("wrow", (SAM,), F32, kind="ExternalInput")
    nb_in = nc.dram_tensor("nb", (112, 4), F32, kind="ExternalInput")

    # out[b, c, p, ih, j] = result[b, c, ih*112 + p, j]
    out_dram = nc.dram_tensor("out", (BSH, 3, 112, 2, SAM), F16,
                              kind="ExternalOutput")

    with tile.TileContext(nc) as tc, ExitStack() as ctx:
        consts = ctx.enter_context(tc.tile_pool(name="consts", bufs=1))
        p1pool = ctx.enter_context(tc.tile_pool(name="p1pool", bufs=1))
        dpool = ctx.enter_context(tc.tile_pool(name="dpool", bufs=2))
        sigpool = ctx.enter_context(tc.tile_pool(name="sigpool", bufs=2))
        apool = ctx.enter_context(tc.tile_pool(name="apool", bufs=12))
        wpool = ctx.enter_context(tc.tile_pool(name="wpool", bufs=2))
        epool = ctx.enter_context(tc.tile_pool(name="epool", bufs=3))
        opool = ctx.enter_context(tc.tile_pool(name="opool", bufs=2))
        ps1 = ctx.enter_context(tc.tile_pool(name="ps1", bufs=2, space="PSUM"))
        psA = ctx.enter_context(tc.tile_pool(name="psA", bufs=3, space="PSUM"))
        psB = ctx.enter_context(tc.tile_pool(name="psB", bufs=2, space="PSUM"))
        psC = ctx.enter_context(tc.tile_pool(name="psC", bufs=1, space="PSUM"))

        # ---------------- constants ----------------
        ident = consts.tile([128, 128], F32)
        masks.make_identity(nc, ident[:])
        # ---------------- all input DMAs ----------------
        # att[0] first (split in cc chunks so its marginals start early),
        # then the small constants (Act queue), then att[1..7] and data.
        att_t = []
        for b in range(BSH):
            a = p1pool.tile([112, 4, IN], F32, tag=f"att{b}", bufs=1,
                            name=f"att_t{b}")
            att_t.append(a)
        att0_src = att_in[0].rearrange("(cc p) x -> p cc x", p=112)
        for cc in range(4):
            nc.sync.dma_start(out=att_t[0][:, cc, :], in_=att0_src[:, cc, :])

        nb = consts.tile([112, 4], F32)
        nc.sync.dma_start(out=nb, in_=nb_in[:, :])
        wrow = consts.tile([16, SAM], F32)
        nc.sync.dma_start(out=wrow, in_=bass.AP(wrow_in, 0, [[0, 16], [1, SAM]]))
        prow = consts.tile([16, 672], F32)
        nc.sync.dma_start(out=prow, in_=bass.AP(prow_in, 0, [[0, 16], [1, 672]]))
        wc = consts.tile([112, 6, SAM], F32)
        nc.sync.dma_start(out=wc, in_=wmat_in.rearrange("(gc p) o -> p gc o", p=112))

        for b in range(1, BSH):
            nc.sync.dma_start(
                out=att_t[b], in_=att_in[b].rearrange("(cc p) x -> p cc x", p=112))
        at_tiles = {}
        for b in range(BSH):
            for c in range(3):
                at = apool.tile([112, 4, IN], F16, tag="at", name=f"at{b}{c}")
                nc.sync.dma_start(
                    out=at, in_=data_in[b, c].rearrange("(cc p) x -> p cc x", p=112))
                at_tiles[(b, c)] = at

        # ---------------- per-group marginals + conv -> coords ----------------
        def emit_group(bs):
            G = len(bs)
            tt_mid = nc.vector.tensor_tensor
            marg = sigpool.tile([112, 4, 8], F32, tag="marg", name="marg")
            for lb, b in enumerate(bs):
                a = att_t[b]
                # y-profile: max over x (free dim), split per cc chunk so
                # the scheduler can interleave critical small ops
                for cc4 in range(4):
                    nc.vector.tensor_reduce(
                        out=marg[:, cc4, G + lb:G + lb + 1], in_=a[:, cc4, :],
                        axis=mybir.AxisListType.X, op=ALU.max)
                # x-profile: fold cc by max (DVE; Pool cannot do max),
                # split in halves for finer scheduling granules
                m1 = dpool.tile([112, IN], F32, tag="m1", name="m1")
                m2 = dpool.tile([112, IN], F32, tag="m2", name="m2")
                for h in range(2):
                    sl = slice(h * 224, (h + 1) * 224)
                    nc.vector.tensor_tensor(
                        out=m1[:, sl], in0=a[:, 0, sl], in1=a[:, 1, sl],
                        op=ALU.max)
                    nc.vector.tensor_tensor(
                        out=m2[:, sl], in0=a[:, 2, sl], in1=a[:, 3, sl],
                        op=ALU.max)
                    nc.vector.tensor_tensor(
                        out=m1[:, sl], in0=m1[:, sl], in1=m2[:, sl],
                        op=ALU.max)
                mt = ps1.tile([112, 4, 112], F32, tag="p1", name="mt")
                for xc in range(4):
                    nc.tensor.transpose(
                        mt[:, xc, :], m1[:, xc * 112:(xc + 1) * 112],
                        ident[0:112, 0:112])
                nc.vector.tensor_reduce(
                    out=marg[:, :, lb], in_=mt, axis=mybir.AxisListType.X,
                    op=ALU.max)

            # rows 0:G = x-profiles, G:2G = y-profiles
            mgps = ps1.tile([8, 4, 112], F32, tag="p1", name="mgps")
            for cc in range(4):
                nc.tensor.transpose(
                    mgps[0:2 * G, cc, :], marg[:, cc, 0:2 * G],
                    ident[0:112, 0:112])
            mg = sigpool.tile([8, 4, 112], F32, tag="mg", name="mg")
            nc.vector.tensor_copy(
                out=_flat(mg, 2 * G, IN), in_=_flat(mgps, 2 * G, IN))

            # No normalization: the profile scale cancels exactly in
            # xf = conv(P*m)/conv(m).
            # linear downsample 448 -> 224 (align_corners), written straight
            # into the signal tile's center: msn = even + (odd - even) * wrow
            even = _flat(mg, 2 * G, SAM, 0, 2)
            odd = _flat(mg, 2 * G, SAM, 1, 2)
            sig = sigpool.tile([8, 672], F32, tag="sig", name="sig")
            sigP = sigpool.tile([8, 672], F32, tag="sigP", name="sigP")
            nc.gpsimd.memset(sig[0:2 * G, 670:672], 0.0)
            nc.gpsimd.memset(sigP[0:2 * G, 670:672], 0.0)
            diff = sigpool.tile([8, SAM], F32, tag="diff", name="diff")
            tt_mid(out=diff[0:2 * G, :], in0=odd, in1=even, op=ALU.subtract)
            tt_mid(out=diff[0:2 * G, :], in0=diff[0:2 * G, :],
                   in1=wrow[0:2 * G, :], op=ALU.mult)
            tt_mid(out=sig[0:2 * G, 223:447], in0=diff[0:2 * G, :], in1=even,
                   op=ALU.add)
            # reflect pads copied from the center (Pool, SBUF->SBUF)
            lpad = bass.AP(sig.tensor, sig.offset + 446,
                           [[sig.ap[0][0], 2 * G], [-1, 223]])
            nc.gpsimd.tensor_copy(out=sig[0:2 * G, 0:223], in_=lpad)
            rpad = bass.AP(sig.tensor, sig.offset + 445,
                           [[sig.ap[0][0], 2 * G], [-1, 223]])
            nc.gpsimd.tensor_copy(out=sig[0:2 * G, 447:670], in_=rpad)
            tt_mid(out=sigP[0:2 * G, 0:670], in0=sig[0:2 * G, 0:670],
                   in1=prow[0:2 * G, 0:670], op=ALU.mult)

            # transpose signals to [g-part, rows]; cols 0:2G sig, 2G:4G sigP
            sigT_ps = ps1.tile([112, 6, 16], F32, tag="p1", name="sigT_ps")
            for gc in range(6):
                nc.tensor.transpose(
                    sigT_ps[:, gc, 0:2 * G],
                    sig[0:2 * G, gc * 112:(gc + 1) * 112], ident[0:2 * G, 0:2 * G])
                nc.tensor.transpose(
                    sigT_ps[:, gc, 2 * G:4 * G],
                    sigP[0:2 * G, gc * 112:(gc + 1) * 112], ident[0:2 * G, 0:2 * G])
            sigT = sigpool.tile([112, 6, 16], F32, tag="sigT", name="sigT")
            nc.vector.tensor_copy(
                out=bass.AP(sigT.tensor, sigT.offset,
                            [[sigT.ap[0][0], 112], [16, 6], [1, 4 * G]]),
                in_=bass.AP(sigT_ps.tensor, sigT_ps.offset,
                            [[sigT_ps.ap[0][0], 112], [16, 6], [1, 4 * G]]))

            # 447-tap conv via Toeplitz matmuls (true fp32)
            px_ps = ps1.tile([112, 2, 16], F32, tag="p1", name="px_ps")
            for oh in range(2):
                for gc in range(6):
                    nc.tensor.matmul(
                        px_ps[:, oh, 0:4 * G],
                        lhsT=wc[:, gc, oh * 112:(oh + 1) * 112],
                        rhs=sigT[:, gc, 0:4 * G],
                        start=(gc == 0), stop=(gc == 5))
            px = sigpool.tile([112, 2, 16], F32, tag="px", name="px")
            nc.vector.tensor_copy(
                out=bass.AP(px.tensor, px.offset,
                            [[px.ap[0][0], 112], [16, 2], [1, 4 * G]]),
                in_=bass.AP(px_ps.tensor, px_ps.offset,
                            [[px_ps.ap[0][0], 112], [16, 2], [1, 4 * G]]))

            # pc = clip(447 * conv(P*m)/conv(m), 0, 447); col r<G: x, r>=G: y
            rec = sigpool.tile([112, 2, 8], F32, tag="rec", name="rec")
            rec_ap = bass.AP(rec.tensor, rec.offset,
                             [[rec.ap[0][0], 112], [8, 2], [1, 2 * G]])
            nc.vector.reciprocal(
                out=rec_ap,
                in_=bass.AP(px.tensor, px.offset,
                            [[px.ap[0][0], 112], [16, 2], [1, 2 * G]]))
            pc = sigpool.tile([112, 2, 8], F32, tag="pc", name="pc")
            pc_ap = bass.AP(pc.tensor, pc.offset,
                            [[pc.ap[0][0], 112], [8, 2], [1, 2 * G]])
            nc.vector.tensor_tensor(
                out=pc_ap,
                in0=bass.AP(px.tensor, px.offset + 2 * G,
                            [[px.ap[0][0], 112], [16, 2], [1, 2 * G]]),
                in1=rec_ap, op=ALU.mult)
            nc.vector.tensor_scalar(
                out=pc_ap, in0=pc_ap, scalar1=447.0, scalar2=0.0,
                op0=ALU.mult, op1=ALU.max)
            nc.vector.tensor_scalar(
                out=pc_ap, in0=pc_ap, scalar1=447.0, scalar2=None,
                op0=ALU.min)
            return pc

        # per-batch tents from pc (lb = index within group). Returns the
        # weight tiles plus a list of closures that emit the actual ops, so
        # the schedule can interleave them into the previous batch's frames
        # (keeps the Act/DVE/Pool queues free of head-of-line bursts).
        def tent_closures(pc, G, lb):
            # w[p, axis, cc, j]: axis 0 = y tents (rhs of stage 1),
            # axis 1 = x tents (rhs of stage 2); both negated (cancels)
            w = wpool.tile([112, 2, 4, SAM], F16, tag="w", name="w")
            state = {}
            ops = []

            def bc_op():
                # bc[:, 0, :] = y-coords broadcast, bc[:, 1, :] = x-coords
                bc = psC.tile([112, 2, SAM], F32, tag="bc", name="bc")
                for ax, r in ((0, G + lb), (1, lb)):
                    for oh in range(2):
                        nc.tensor.transpose(
                            bc[:, ax, oh * 112:(oh + 1) * 112],
                            _bcast_lhsT(pc, oh * 8 + r), ident[0:112, 0:112])
                state["bc"] = bc

            ops.append(bc_op)
            # y tents first: stage-1's accumulation chain consumes them in
            # cc order, x tents only gate stage-2
            for ax in range(2):
                for cc in range(4):
                    def pair(ax=ax, cc=cc):
                        bc = state["bc"]
                        a2 = sigpool.tile([112, SAM], F32, tag="arm", bufs=3,
                                          name="a2")
                        nc.scalar.activation(
                            out=a2, in_=bc[:, ax, :], func=ACTF.Abs,
                            bias=nb[:, cc:cc + 1], scale=1.0)
                        nc.gpsimd.tensor_scalar(
                            out=w[:, ax, cc, :], in0=a2, scalar1=1.0,
                            scalar2=0.0, op0=ALU.subtract, op1=ALU.min)
                    ops.append(pair)
            return w, ops

        def emit_frames(b, w, side):
            late_b = b >= 4
            osb = opool.tile([112, 3, 2, SAM], F16, tag="osb", name="osb")
            for c in range(3):
                at = at_tiles[(b, c)]
                bt = epool.tile([112, 4, SAM], F16, tag="bt", name="bt")
                for q in range(2):
                    psa = psA.tile([112, 2, SAM], F32, tag="psa", name="psa")
                    for k2 in range(2):
                        xc = 2 * q + k2
                        for yc in range(4):
                            nc.tensor.matmul(
                                psa[:, k2, :],
                                lhsT=at[:, yc, xc * 112:(xc + 1) * 112],
                                rhs=w[:, 0, yc, :],
                                start=(yc == 0), stop=(yc == 3))
                    if late_b and q == 0:
                        nc.vector.tensor_copy(out=bt[:, 0:2, :], in_=psa)
                    else:
                        nc.scalar.copy(out=bt[:, 2 * q:2 * q + 2, :], in_=psa)
                if side and c == 1:
                    side.popleft()()
                psb = psB.tile([112, 2, SAM], F32, tag="psb", name="psb")
                for ih in range(2):
                    for xc in range(4):
                        nc.tensor.matmul(
                            psb[:, ih, :],
                            lhsT=bt[:, xc, ih * 112:(ih + 1) * 112],
                            rhs=w[:, 1, xc, :],
                            start=(xc == 0), stop=(xc == 3))
                if b == BSH - 1 and c == 2:
                    # final frame: copy halves on two engines in parallel
                    nc.vector.tensor_copy(out=osb[:, c, 0:1, :],
                                          in_=psb[:, 0:1, :])
                    nc.scalar.copy(out=osb[:, c, 1:2, :], in_=psb[:, 1:2, :])
                elif late_b:
                    nc.vector.tensor_copy(out=osb[:, c, :, :], in_=psb)
                else:
                    nc.scalar.copy(out=osb[:, c, :, :], in_=psb)
                if b == BSH - 1:
                    # split the final batch's writes per frame: shortens the
                    # end-of-program tail; final write on the idle sync queue
                    wq = nc.sync if c == 2 else nc.scalar
                    wq.dma_start(out=out_dram[b, c], in_=osb[:, c, :, :])
                # drain next batch's tent ops, spread over this batch's frames
                take = (len(side) + 2 - c) // (3 - c)
                for _ in range(take):
                    side.popleft()()
            if b < BSH - 1:
                nc.scalar.dma_start(
                    out=out_dram[b].rearrange("c p ih j -> p c ih j"),
                    in_=osb)

        # ---------------- interleaved schedule ----------------
        from collections import deque

        group_of = {b: gi for gi, g in enumerate(GROUPS) for b in g}
        lb_of = {b: g.index(b) for g in GROUPS for b in g}
        pcs = {}

        def ensure_group(gi):
            if gi not in pcs:
                pcs[gi] = emit_group(GROUPS[gi])

        w = {}
        ensure_group(0)
        w[0], ops0 = tent_closures(pcs[0], 1, 0)
        for op in ops0:
            op()
        for b in range(BSH):
            nxt = b + 1
            side = deque()
            if nxt < BSH:
                gi = group_of[nxt]
                ensure_group(gi)
                w[nxt], opsn = tent_closures(pcs[gi], len(GROUPS[gi]),
                                             lb_of[nxt])
                side = deque(opsn)
            emit_frames(b, w[b], side)

    nc.compile()
    return nc


def _static_consts(filter_w: np.ndarray):
    # Toeplitz layout of the (zero-padded) filter: wmat[g, o] = wpad[223+g-o]
    wpad = np.zeros(896, dtype=np.float32)
    wpad[223:223 + KSIZE] = filter_w
    g = np.arange(672)
    o = np.arange(SAM)
    idx = 223 + g[:, None] - o[None, :]
    valid = (idx >= 0) & (idx < 896)
    wmat = np.zeros((672, SAM), dtype=np.float32)
    wmat[valid] = wpad[idx[valid]]

    prow = np.zeros(672, dtype=np.float32)
    prow[0:GLOB] = (np.arange(GLOB, dtype=np.float32) - PAD) / (SAM - 1.0)
    wrow = (np.arange(SAM, dtype=np.float32) / float(PAD)).astype(np.float32)
    base = (np.arange(112, dtype=np.float32)[:, None]
            + 112.0 * np.arange(4, dtype=np.float32)[None, :])
    nb = (-base).astype(np.float32)
    return {"wmat": wmat, "prow": prow, "wrow": wrow, "nb": nb}


def kernel(data: np.ndarray, structure_att: np.ndarray,
           filter_w: np.ndarray) -> np.ndarray:
    global last_results
    data16 = np.ascontiguousarray(data, dtype=np.float16)
    structure_att = np.ascontiguousarray(structure_att, dtype=np.float32)
    filter_w = np.ascontiguousarray(filter_w, dtype=np.float32)

    if "nc" not in _CACHE:
        _CACHE["nc"] = _build_program()
    nc = _CACHE["nc"]

    consts = _static_consts(filter_w)
    in_maps = []
    for core in range(NCORES):
        sl = slice(core * BSH, (core + 1) * BSH)
        in_maps.append({
            "data": data16[sl], "att": structure_att[sl], **consts,
        })

    res = run_bass_kernel_spmd(nc, in_maps, core_ids=list(range(NCORES)))
    last_results = res
    parts = []
    for i in range(NCORES):
        o = res.results[i]["out"]  # [BSH, 3, 112, 2, 224] fp16
        parts.append(np.transpose(o, (0, 1, 3, 2, 4)).reshape(BSH, 3, SAM, SAM))
    return np.concatenate(parts, axis=0).astype(np.float32)


# revision 32
# speedup vs baseline: 1.5473x; 1.0045x over previous
"""Trainium2 Bass kernel for nn_MASNET2 (structure-attention warped resampling).

Pipeline per batch:
  1. axis-max marginals of structure_att  -> x/y profiles            (f32)
  2. normalize, linear-downsample 448->224, reflect-pad to 670       (f32)
  3. 447-tap conv (plain + coordinate-weighted) -> sampling coords   (f32)
  4. separable bilinear grid-sample of data via two tent matmuls     (fp16)

Sharding: pure data-parallel, batch 64 -> 8 cores x 8.

Key implementation choices:
  - data / tent weights / intermediates / output staged in fp16: halves the
    HBM traffic of the dominant streams and runs the PE at 1 cyc/row with
    no moving-dim padding (N=224). Coordinates stay f32 end-to-end.
  - tents are negated: w = min(|coord - base| - 1, 0) = -tent, one Abs
    (Act engine) + one tensor_scalar (Pool) per arm; the negations cancel
    across the two matmul stages. The profile normalization is dropped
    entirely (it cancels exactly in conv(P*m)/conv(m)).
  - coordinate broadcast partition->free via a single PE transpose with a
    stride-0 free-dim access pattern (no DRAM round-trip).
  - marginal/conv chain is split into batch groups so the first batch's
    sampling grid is ready ~16us in and the PE pipeline starts while the
    attention/data DMA stream is still running; later groups' marginals
    hide under the frame pipeline.
  - input DMAs are ordered so early batches' data arrives right behind
    their attention frames; PSUM->SBUF copies are spread across DVE/Act
    (GPSIMD cannot touch PSUM), tent arms across Act/Pool.
"""
import sys

sys.path.insert(0, "/opt/trn_rl_repo")

import numpy as np
from contextlib import ExitStack

import concourse.bass as bass
import concourse.bacc as bacc
import concourse.tile as tile
from concourse import mybir, masks
from concourse.bass_utils import run_bass_kernel_spmd

F32 = mybir.dt.float32
F16 = mybir.dt.float16
ALU = mybir.AluOpType
ACTF = mybir.ActivationFunctionType

SAM = 224
IN = 448
PAD = 223
GLOB = 670
KSIZE = 447
NCORES = 8
BSH = 8  # batch shard per core
GROUPS = [[0], [1], [2], [3, 4], [5, 6], [7]]

_CACHE = {}

# expose the last run's results for test.py profiling
last_results = None


def _flat(t, p_cnt, free_cnt, extra_off=0, stride=1):
    """2D view [p_cnt, free_cnt] of a tile's storage (custom free AP)."""
    return bass.AP(t.tensor, t.offset + extra_off,
                   [[t.ap[0][0], p_cnt], [stride, free_cnt]])


def _bcast_lhsT(t, extra_off, n=112):
    """stride-0 free-dim AP: lhsT[k, m] = t[k]@extra_off for all m."""
    return bass.AP(t.tensor, t.offset + extra_off,
                   [[t.ap[0][0], 112], [0, n]])


def _build_program():
    nc = bacc.Bacc("TRN2", num_devices=NCORES)

    data_in = nc.dram_tensor("data", (BSH, 3, IN, IN), F16, kind="ExternalInput")
    att_in = nc.dram_tensor("att", (BSH, IN, IN), F32, kind="ExternalInput")
    wmat_in = nc.dram_tensor("wmat", (672, SAM), F32, kind="ExternalInput")
    prow_in = nc.dram_tensor("prow", (672,), F32, kind="ExternalInput")
    wrow_in = nc.dram_tensor("wrow", (SAM,), F32, kind="ExternalInput")
    nb_in = nc.dram_tensor("nb", (112, 4), F32, kind="ExternalInput")

    # out[b, c, p, ih, j] = result[b, c, ih*112 + p, j]
    out_dram = nc.dram_tensor("out", (BSH, 3, 112, 2, SAM), F16,
                              kind="ExternalOutput")

    with tile.TileContext(nc) as tc, ExitStack() as ctx:
        consts = ctx.enter_context(tc.tile_pool(name="consts", bufs=1))
        p1pool = ctx.enter_context(tc.tile_pool(name="p1pool", bufs=1))
        dpool = ctx.enter_context(tc.tile_pool(name="dpool", bufs=2))
        sigpool = ctx.enter_context(tc.tile_pool(name="sigpool", bufs=2))
        apool = ctx.enter_context(tc.tile_pool(name="apool", bufs=12))
        wpool = ctx.enter_context(tc.tile_pool(name="wpool", bufs=2))
        epool = ctx.enter_context(tc.tile_pool(name="epool", bufs=3))
        opool = ctx.enter_context(tc.tile_pool(name="opool", bufs=2))
        ps1 = ctx.enter_context(tc.tile_pool(name="ps1", bufs=2, space="PSUM"))
        psA = ctx.enter_context(tc.tile_pool(name="psA", bufs=3, space="PSUM"))
        psB = ctx.enter_context(tc.tile_pool(name="psB", bufs=2, space="PSUM"))
        psC = ctx.enter_context(tc.tile_pool(name="psC", bufs=1, space="PSUM"))

        # ---------------- constants ----------------
        ident = consts.tile([128, 128], F32)
        masks.make_identity(nc, ident[:])
        # ---------------- all input DMAs ----------------
        # att[0] first (split in cc chunks so its marginals start early),
        # then the small constants (Act queue), then att[1..7] and data.
        att_t = []
        for b in range(BSH):
            a = p1pool.tile([112, 4, IN], F32, tag=f"att{b}", bufs=1,
                            name=f"att_t{b}")
            att_t.append(a)
        att0_src = att_in[0].rearrange("(cc p) x -> p cc x", p=112)
        for cc in range(4):
            nc.sync.dma_start(out=att_t[0][:, cc, :], in_=att0_src[:, cc, :])

        nb = consts.tile([112, 4], F32)
        nc.sync.dma_start(out=nb, in_=nb_in[:, :])
        wrow = consts.tile([16, SAM], F32)
        nc.sync.dma_start(out=wrow, in_=bass.AP(wrow_in, 0, [[0, 16], [1, SAM]]))
        prow = consts.tile([16, 672], F32)
        nc.sync.dma_start(out=prow, in_=bass.AP(prow_in, 0, [[0, 16], [1, 672]]))
        wc = consts.tile([112, 6, SAM], F32)
        nc.sync.dma_start(out=wc, in_=wmat_in.rearrange("(gc p) o -> p gc o", p=112))

        for b in range(1, BSH):
            nc.sync.dma_start(
                out=att_t[b], in_=att_in[b].rearrange("(cc p) x -> p cc x", p=112))
        at_tiles = {}
        for b in range(BSH):
            for c in range(3):
                at = apool.tile([112, 4, IN], F16, tag="at", name=f"at{b}{c}")
                nc.sync.dma_start(
                    out=at, in_=data_in[b, c].rearrange("(cc p) x -> p cc x", p=112))
                at_tiles[(b, c)] = at

        # ---------------- per-group marginals + conv -> coords ----------------
        def emit_group(bs):
            G = len(bs)
            tt_mid = nc.vector.tensor_tensor
            marg = sigpool.tile([112, 4, 8], F32, tag="marg", name="marg")
            for lb, b in enumerate(bs):
                a = att_t[b]
                # y-profile: max over x (free dim), split per cc chunk so
                # the scheduler can interleave critical small ops
                for cc4 in range(4):
                    nc.vector.tensor_reduce(
                        out=marg[:, cc4, G + lb:G + lb + 1], in_=a[:, cc4, :],
                        axis=mybir.AxisListType.X, op=ALU.max)
                # x-profile: fold cc by max (DVE; Pool cannot do max),
                # split in halves for finer scheduling granules
                m1 = dpool.tile([112, IN], F32, tag="m1", name="m1")
                m2 = dpool.tile([112, IN], F32, tag="m2", name="m2")
                for h in range(2):
                    sl = slice(h * 224, (h + 1) * 224)
                    nc.vector.tensor_tensor(
                        out=m1[:, sl], in0=a[:, 0, sl], in1=a[:, 1, sl],
                        op=ALU.max)
                    nc.vector.tensor_tensor(
                        out=m2[:, sl], in0=a[:, 2, sl], in1=a[:, 3, sl],
                        op=ALU.max)
                    nc.vector.tensor_tensor(
                        out=m1[:, sl], in0=m1[:, sl], in1=m2[:, sl],
                        op=ALU.max)
                mt = ps1.tile([112, 4, 112], F32, tag="p1", name="mt")
                for xc in range(4):
                    nc.tensor.transpose(
                        mt[:, xc, :], m1[:, xc * 112:(xc + 1) * 112],
                        ident[0:112, 0:112])
                nc.vector.tensor_reduce(
                    out=marg[:, :, lb], in_=mt, axis=mybir.AxisListType.X,
                    op=ALU.max)

            # rows 0:G = x-profiles, G:2G = y-profiles
            mgps = ps1.tile([8, 4, 112], F32, tag="p1", name="mgps")
            for cc in range(4):
                nc.tensor.transpose(
                    mgps[0:2 * G, cc, :], marg[:, cc, 0:2 * G],
                    ident[0:112, 0:112])
            mg = sigpool.tile([8, 4, 112], F32, tag="mg", name="mg")
            nc.vector.tensor_copy(
                out=_flat(mg, 2 * G, IN), in_=_flat(mgps, 2 * G, IN))

            # No normalization: the profile scale cancels exactly in
            # xf = conv(P*m)/conv(m).
            # linear downsample 448 -> 224 (align_corners), written straight
            # into the signal tile's center: msn = even + (odd - even) * wrow
            even = _flat(mg, 2 * G, SAM, 0, 2)
            odd = _flat(mg, 2 * G, SAM, 1, 2)
            sig = sigpool.tile([8, 672], F32, tag="sig", name="sig")
            sigP = sigpool.tile([8, 672], F32, tag="sigP", name="sigP")
            nc.gpsimd.memset(sig[0:2 * G, 670:672], 0.0)
            nc.gpsimd.memset(sigP[0:2 * G, 670:672], 0.0)
            diff = sigpool.tile([8, SAM], F32, tag="diff", name="diff")
            tt_mid(out=diff[0:2 * G, :], in0=odd, in1=even, op=ALU.subtract)
            tt_mid(out=diff[0:2 * G, :], in0=diff[0:2 * G, :],
                   in1=wrow[0:2 * G, :], op=ALU.mult)
            tt_mid(out=sig[0:2 * G, 223:447], in0=diff[0:2 * G, :], in1=even,
                   op=ALU.add)
            # reflect pads copied from the center (Pool, SBUF->SBUF)
            lpad = bass.AP(sig.tensor, sig.offset + 446,
                           [[sig.ap[0][0], 2 * G], [-1, 223]])
            nc.gpsimd.tensor_copy(out=sig[0:2 * G, 0:223], in_=lpad)
            rpad = bass.AP(sig.tensor, sig.offset + 445,
                           [[sig.ap[0][0], 2 * G], [-1, 223]])
            nc.gpsimd.tensor_copy(out=sig[0:2 * G, 447:670], in_=rpad)
            tt_mid(out=sigP[0:2 * G, 0:670], in0=sig[0:2 * G, 0:670],
                   in1=prow[0:2 * G, 0:670], op=ALU.mult)

            # transpose signals to [g-part, rows]; cols 0:2G sig, 2G:4G sigP
            sigT_ps = ps1.tile([112, 6, 16], F32, tag="p1", name="sigT_ps")
            for gc in range(6):
                nc.tensor.transpose(
                    sigT_ps[:, gc, 0:2 * G],
                    sig[0:2 * G, gc * 112:(gc + 1) * 112], ident[0:2 * G, 0:2 * G])
                nc.tensor.transpose(
                    sigT_ps[:, gc, 2 * G:4 * G],
                    sigP[0:2 * G, gc * 112:(gc + 1) * 112], ident[0:2 * G, 0:2 * G])
            sigT = sigpool.tile([112, 6, 16], F32, tag="sigT", name="sigT")
            nc.vector.tensor_copy(
                out=bass.AP(sigT.tensor, sigT.offset,
                            [[sigT.ap[0][0], 112], [16, 6], [1, 4 * G]]),
                in_=bass.AP(sigT_ps.tensor, sigT_ps.offset,
                            [[sigT_ps.ap[0][0], 112], [16, 6], [1, 4 * G]]))

            # 447-tap conv via Toeplitz matmuls (true fp32)
            px_ps = ps1.tile([112, 2, 16], F32, tag="p1", name="px_ps")
            for oh in range(2):
                for gc in range(6):
                    nc.tensor.matmul(
                        px_ps[:, oh, 0:4 * G],
                        lhsT=wc[:, gc, oh * 112:(oh + 1) * 112],
                        rhs=sigT[:, gc, 0:4 * G],
                        start=(gc == 0), stop=(gc == 5))
            px = sigpool.tile([112, 2, 16], F32, tag="px", name="px")
            nc.vector.tensor_copy(
                out=bass.AP(px.tensor, px.offset,
                            [[px.ap[0][0], 112], [16, 2], [1, 4 * G]]),
                in_=bass.AP(px_ps.tensor, px_ps.offset,
                            [[px_ps.ap[0][0], 112], [16, 2], [1, 4 * G]]))

            # pc = clip(447 * conv(P*m)/conv(m), 0, 447); col r<G: x, r>=G: y
            rec = sigpool.tile([112, 2, 8], F32, tag="rec", name="rec")
            rec_ap = bass.AP(rec.tensor, rec.offset,
                             [[rec.ap[0][0], 112], [8, 2], [1, 2 * G]])
            nc.vector.reciprocal(
                out=rec_ap,
                in_=bass.AP(px.tensor, px.offset,
                            [[px.ap[0][0], 112], [16, 2], [1, 2 * G]]))
            pc = sigpool.tile([112, 2, 8], F32, tag="pc", name="pc")
            pc_ap = bass.AP(pc.tensor, pc.offset,
                            [[pc.ap[0][0], 112], [8, 2], [1, 2 * G]])
            nc.vector.tensor_tensor(
                out=pc_ap,
                in0=bass.AP(px.tensor, px.offset + 2 * G,
                            [[px.ap[0][0], 112], [16, 2], [1, 2 * G]]),
                in1=rec_ap, op=ALU.mult)
            nc.vector.tensor_scalar(
                out=pc_ap, in0=pc_ap, scalar1=447.0, scalar2=0.0,
                op0=ALU.mult, op1=ALU.max)
            nc.vector.tensor_scalar(
                out=pc_ap, in0=pc_ap, scalar1=447.0, scalar2=None,
                op0=ALU.min)
            return pc

        # per-batch tents from pc (lb = index within group). Returns the
        # weight tiles plus a list of closures that emit the actual ops, so
        # the schedule can interleave them into the previous batch's frames
        # (keeps the Act/DVE/Pool queues free of head-of-line bursts).
        def tent_closures(pc, G, lb):
            # w[p, axis, cc, j]: axis 0 = y tents (rhs of stage 1),
            # axis 1 = x tents (rhs of stage 2); both negated (cancels)
            w = wpool.tile([112, 2, 4, SAM], F16, tag="w", name="w")
            state = {}
            ops = []

            def bc_op():
                # bc[:, 0, :] = y-coords broadcast, bc[:, 1, :] = x-coords
                bc = psC.tile([112, 2, SAM], F32, tag="bc", name="bc")
                for ax, r in ((0, G + lb), (1, lb)):
                    for oh in range(2):
                        nc.tensor.transpose(
                            bc[:, ax, oh * 112:(oh + 1) * 112],
                            _bcast_lhsT(pc, oh * 8 + r), ident[0:112, 0:112])
                state["bc"] = bc

            ops.append(bc_op)
            # y tents first: stage-1's accumulation chain consumes them in
            # cc order, x tents only gate stage-2
            for ax in range(2):
                for cc in range(4):
                    def pair(ax=ax, cc=cc):
                        bc = state["bc"]
                        a2 = sigpool.tile([112, SAM], F32, tag="arm", bufs=3,
                                          name="a2")
                        nc.scalar.activation(
                            out=a2, in_=bc[:, ax, :], func=ACTF.Abs,
                            bias=nb[:, cc:cc + 1], scale=1.0)
                        nc.gpsimd.tensor_scalar(
                            out=w[:, ax, cc, :], in0=a2, scalar1=1.0,
                            scalar2=0.0, op0=ALU.subtract, op1=ALU.min)
                    ops.append(pair)
            return w, ops

        def emit_frames(b, w, side):
            late_b = b >= 4
            osb = opool.tile([112, 3, 2, SAM], F16, tag="osb", name="osb")
            for c in range(3):
                at = at_tiles[(b, c)]
                bt = epool.tile([112, 4, SAM], F16, tag="bt", name="bt")
                for q in range(2):
                    psa = psA.tile([112, 2, SAM], F32, tag="psa", name="psa")
                    for k2 in range(2):
                        xc = 2 * q + k2
                        for yc in range(4):
                            nc.tensor.matmul(
                                psa[:, k2, :],
                                lhsT=at[:, yc, xc * 112:(xc + 1) * 112],
                                rhs=w[:, 0, yc, :],
                                start=(yc == 0), stop=(yc == 3))
                    if (late_b or c == 1) and q == 0:
                        nc.vector.tensor_copy(out=bt[:, 0:2, :], in_=psa)
                    else:
                        nc.scalar.copy(out=bt[:, 2 * q:2 * q + 2, :], in_=psa)
                if side and c == 1:
                    side.popleft()()
                psb = psB.tile([112, 2, SAM], F32, tag="psb", name="psb")
                for ih in range(2):
                    for xc in range(4):
                        nc.tensor.matmul(
                            psb[:, ih, :],
                            lhsT=bt[:, xc, ih * 112:(ih + 1) * 112],
                            rhs=w[:, 1, xc, :],
                            start=(xc == 0), stop=(xc == 3))
                if b == BSH - 1 and c == 2:
                    # final frame: copy halves on two engines in parallel
                    nc.vector.tensor_copy(out=osb[:, c, 0:1, :],
                                          in_=psb[:, 0:1, :])
                    nc.scalar.copy(out=osb[:, c, 1:2, :], in_=psb[:, 1:2, :])
                elif late_b:
                    nc.vector.tensor_copy(out=osb[:, c, :, :], in_=psb)
                else:
                    nc.scalar.copy(out=osb[:, c, :, :], in_=psb)
                if b == BSH - 1:
                    # split the final batch's writes per frame: shortens the
                    # end-of-program tail; final write on the idle sync queue
                    nc.sync.dma_start(out=out_dram[b, c], in_=osb[:, c, :, :])
                # drain next batch's tent ops, spread over this batch's frames
                take = (len(side) + 2 - c) // (3 - c)
                for _ in range(take):
                    side.popleft()()
            if b < BSH - 1:
                wq2 = nc.sync if b == 6 else nc.scalar
                wq2.dma_start(
                    out=out_dram[b].rearrange("c p ih j -> p c ih j"),
                    in_=osb)

        # ---------------- interleaved schedule ----------------
        from collections import deque

        group_of = {b: gi for gi, g in enumerate(GROUPS) for b in g}
        lb_of = {b: g.index(b) for g in GROUPS for b in g}
        pcs = {}

        def ensure_group(gi):
            if gi not in pcs:
                pcs[gi] = emit_group(GROUPS[gi])

        w = {}
        ensure_group(0)
        w[0], ops0 = tent_closures(pcs[0], 1, 0)
        for op in ops0:
            op()
        for b in range(BSH):
            nxt = b + 1
            side = deque()
            if nxt < BSH:
                gi = group_of[nxt]
                ensure_group(gi)
                w[nxt], opsn = tent_closures(pcs[gi], len(GROUPS[gi]),
                                             lb_of[nxt])
                side = deque(opsn)
            emit_frames(b, w[b], side)

    nc.compile()
    return nc


def _static_consts(filter_w: np.ndarray):
    # Toeplitz layout of the (zero-padded) filter: wmat[g, o] = wpad[223+g-o]
    wpad = np.zeros(896, dtype=np.float32)
    wpad[223:223 + KSIZE] = filter_w
    g = np.arange(672)
    o = np.arange(SAM)
    idx = 223 + g[:, None] - o[None, :]
    valid = (idx >= 0) & (idx < 896)
    wmat = np.zeros((672, SAM), dtype=np.float32)
    wmat[valid] = wpad[idx[valid]]

    prow = np.zeros(672, dtype=np.float32)
    prow[0:GLOB] = (np.arange(GLOB, dtype=np.float32) - PAD) / (SAM - 1.0)
    wrow = (np.arange(SAM, dtype=np.float32) / float(PAD)).astype(np.float32)
    base = (np.arange(112, dtype=np.float32)[:, None]
            + 112.0 * np.arange(4, dtype=np.float32)[None, :])
    nb = (-base).astype(np.float32)
    return {"wmat": wmat, "prow": prow, "wrow": wrow, "nb": nb}


def kernel(data: np.ndarray, structure_att: np.ndarray,
           filter_w: np.ndarray) -> np.ndarray:
    global last_results
    data16 = np.ascontiguousarray(data, dtype=np.float16)
    structure_att = np.ascontiguousarray(structure_att, dtype=np.float32)
    filter_w = np.ascontiguousarray(filter_w, dtype=np.float32)

    if "nc" not in _CACHE:
        _CACHE["nc"] = _build_program()
    nc = _CACHE["nc"]

    consts = _static_consts(filter_w)
    in_maps = []
    for core in range(NCORES):
        sl = slice(core * BSH, (core + 1) * BSH)
        in_maps.append({
            "data": data16[sl], "att": structure_att[sl], **consts,
        })

    res = run_bass_kernel_spmd(nc, in_maps, core_ids=list(range(NCORES)))
    last_results = res
    parts = []
    for i in range(NCORES):
        o = res.results[i]["out"]  # [BSH, 3, 112, 2, 224] fp16
        parts.append(np.transpose(o, (0, 1, 3, 2, 4)).reshape(BSH, 3, SAM, SAM))
    return np.concatenate(parts, axis=0).astype(np.float32)
